# revision 47
# baseline (speedup 1.0000x reference)
"""GCN message-passing layer (4x GCNConv sum) on 8 Trainium2 NeuronCores.

out[d] = sum_i ( segment_sum_{e in E_i, dst=d} x[src_e] ) @ W_i

Raw-block SPMD kernel (no Tile scheduler): destination nodes sharded across
8 cores, x replicated. Per core:
  - Host groups edges by (core, set, dst-tile(128), src-bank(32768)), sorts
    each group by src (DRAM locality), pads groups to a cross-core-uniform
    multiple of 128 (pad: src=0, dstf=-1).
  - gpsimd dma_gather (int16 bank-local idxs) stages x[src] rows (bf16) into
    SBUF, round-robin over 4 SWDGE queues (4x descriptor throughput).
  - Processing is set-major within each 4-tile span: while PE consumes set
    i's staging, gpsimd prefetches sets i+1, i+2 (stg ring of 3).
  - DVE builds one-hot S[e,d] = (dstf[e]==d) per 128-edge chunk; TensorE
    accumulates g_i^T = stg^T @ S into per-tile PSUM (1 bank, 6-slot ring).
  - ACT copies tile PSUM->SBUF; TensorE phase 2: out_tile = sum_i gsb_i^T.T
    @ W_i into opsum; ACT copies opsum->SBUF; SP DMAs out rows.
All cross-engine sync is explicit semaphores; every core runs an identical
program (counts are cross-core uniform by construction).
"""
import math
import sys
from contextlib import ExitStack

sys.path.insert(0, "/opt/trn_rl_repo")

import numpy as np
import ml_dtypes

from concourse import bass, mybir, bacc
from concourse.bass_utils import run_bass_kernel_spmd
from concourse.library_config import mlp

P = 128
N_CORES = 8
DT = 128           # dst-tile width (one PSUM bank per (tile, 4 sets))
BANK_ROWS = 32768
NS = 4
GSPAN = 6          # tiles per idx/gather span (must be <= GPSLOTS)
CALL_COLS = 96     # max 128-edge chunks per dma_gather call (12288 idxs)
SRING = 16         # one-hot ring slots
STGRING = 3        # stg (span,set) ring slots
GPSLOTS = 6        # gpsum PSUM rotation (1 bank each)
OTSLOTS = 4        # opsum/ot rotation


class _Cfg:
    def __init__(self, n_nodes):
        self.n_nodes = n_nodes
        self.npc = n_nodes // N_CORES
        self.nt = math.ceil(self.npc / DT)
        self.n_banks = math.ceil(n_nodes / BANK_ROWS)
        self.x_rows_pad = self.n_banks * BANK_ROWS
        self.n_spans = math.ceil(self.nt / GSPAN)


def _host_prep(cfg, edges_list):
    NC, NT, NB = N_CORES, cfg.nt, cfg.n_banks
    counts = np.zeros((NC, NS, NT, NB), np.int64)
    per_set = []
    for i, e in enumerate(edges_list):
        src = np.asarray(e[0], np.int64)
        dst = np.asarray(e[1], np.int64)
        core = dst // cfg.npc
        dloc = dst % cfg.npc
        t = dloc // DT
        b = src // BANK_ROWS
        key = (core * NT + t) * NB + b
        counts[:, i] = np.bincount(key, minlength=NC * NT * NB).reshape(NC, NT, NB)
        order = np.lexsort((src, key))  # src-sorted within group: DRAM locality
        per_set.append((src[order], dloc[order], key[order]))

    C = -(-counts.max(axis=0) // P)
    # guarantee no (set, tile) is entirely empty (phase-2 reads its psum)
    for i in range(NS):
        for t in range(NT):
            if C[i, t].sum() == 0:
                C[i, t, 0] = 1

    col_of = np.zeros((NS, NT, NB), np.int64)
    unit_cols = {}
    span_col0, span_cols = [], []
    col = 0
    for s in range(cfg.n_spans):
        ts = range(s * GSPAN, min((s + 1) * GSPAN, NT))
        span_col0.append(col)
        for i in range(NS):
            for b in range(NB):
                for t in ts:
                    col_of[i, t, b] = col
                    col += C[i, t, b]
                unit_cols[(s, i, b)] = int(C[i, list(ts), b].sum())
        span_cols.append(col - span_col0[-1])
    totc = col
    tot_slots = totc * P

    idx_mats, dstf_mats = [], []
    for c in range(NC):
        slot_src = np.zeros(tot_slots, np.int64)
        slot_dstf = np.full(tot_slots, -1.0, np.float32)
        for i in range(NS):
            src_s, dloc_s, key_s = per_set[i]
            lo = np.searchsorted(key_s, c * NT * NB)
            hi = np.searchsorted(key_s, (c + 1) * NT * NB)
            src_c, dloc_c, key_c = src_s[lo:hi], dloc_s[lo:hi], key_s[lo:hi]
            t_c = (key_c // NB) % NT
            b_c = key_c % NB
            gstart = np.searchsorted(key_c, key_c)
            rank = np.arange(len(key_c)) - gstart
            slot = col_of[i, t_c, b_c] * P + rank
            slot_src[slot] = src_c - b_c * BANK_ROWS
            slot_dstf[slot] = (dloc_c - t_c * DT).astype(np.float32)
        idx16 = slot_src.reshape(tot_slots // 16, 16).T.astype(np.int16)
        idx_mats.append(np.tile(idx16, (8, 1)))
        dstf_mats.append(slot_dstf.reshape(totc, P).T.copy())

    # gather call list: per (s,i,b), split into <=CALL_COLS chunks
    calls = []
    for s in range(cfg.n_spans):
        for i in range(NS):
            for b in range(NB):
                cols = unit_cols[(s, i, b)]
                if cols == 0:
                    continue
                g0 = int(col_of[i, s * GSPAN, b])
                for c0 in range(0, cols, CALL_COLS):
                    cw = min(CALL_COLS, cols - c0)
                    calls.append(dict(s=s, i=i, b=b, col0=g0 + c0, cols=cw))
    # queue = ci % 4 (engine parallelism); completion sem = rotating ring of
    # NGSEM per-call sems (exact completion tracking: cumulative multi-queue
    # counts are unsound because the 16 SDMA engines drain independently).
    NGSEM = 8
    slot_rounds = [0] * NGSEM
    for ci, cl in enumerate(calls):
        cl["q"] = ci % 4
        slot = ci % NGSEM
        slot_rounds[slot] += 1
        cl["slot"] = slot
        cl["round"] = slot_rounds[slot]  # 1-based within rep
    group_waits = {}    # (s,i,b) -> [(slot, round)] for its calls
    span_waits = {}     # s -> {slot: max round} through end of span
    run = {}
    for cl in calls:
        g = (cl["s"], cl["i"], cl["b"])
        group_waits.setdefault(g, []).append((cl["slot"], cl["round"]))
        run[cl["slot"]] = cl["round"]
        span_waits[cl["s"]] = dict(run)

    return dict(C=C, col_of=col_of, unit_cols=unit_cols, totc=totc,
                span_col0=span_col0, span_cols=span_cols,
                idx_mats=idx_mats, dstf_mats=dstf_mats,
                calls=calls, group_waits=group_waits,
                span_waits=span_waits, slot_tot=slot_rounds, ngsem=NGSEM)


def _schedule(cfg, prep):
    """Static per-core schedule: ordered chunk list (set-major within span)
    + per-tile completion chunk counts."""
    C, col_of = prep["C"], prep["col_of"]
    NT, NB = cfg.nt, cfg.n_banks
    chunks = []
    tile_chunk_end = {}   # tile t -> chunk count (1-based) at its completion
    chunks_thru_span = {}
    for s in range(cfg.n_spans):
        ts = range(s * GSPAN, min((s + 1) * GSPAN, NT))
        for i in range(NS):
            for t in ts:
                nib = [(b, int(C[i, t, b])) for b in range(NB) if C[i, t, b] > 0]
                tot_k = sum(n for _, n in nib)
                kk = 0
                for b, cc in nib:
                    for k in range(cc):
                        chunks.append(dict(
                            s=s, t=t, i=i, b=b,
                            col=int(col_of[i, t, b]) + k,
                            start=(kk == 0), stop=(kk == tot_k - 1),
                            first_of_group=(k == 0),
                            last_of_pair=False,
                        ))
                        kk += 1
                if i == NS - 1:
                    tile_chunk_end[t] = len(chunks)
        chunks_thru_span[s] = len(chunks)
    seen = {}
    for ci, ch in enumerate(chunks):
        seen[(ch["s"], ch["i"])] = ci
    for (s, i), ci in seen.items():
        chunks[ci]["last_of_pair"] = True
    return chunks, tile_chunk_end, chunks_thru_span


def _build_kernel(cfg, prep, reps=1, hw_relax=True):
    # hw_relax: skip race-detector-compliance self-waits on the gather
    # queues (within a SWDGE queue, per-engine rings drain FIFO, so the
    # cumulative per-queue sem counts are sound on HW without them).
    NT, NB = cfg.nt, cfg.n_banks
    calls = prep["calls"]
    group_waits = prep["group_waits"]
    span_waits = prep["span_waits"]
    slot_tot = prep["slot_tot"]
    NGSEM = prep["ngsem"]
    span_col0, span_cols = prep["span_col0"], prep["span_cols"]
    unit_cols = prep["unit_cols"]
    col_of = prep["col_of"]
    totc = prep["totc"]
    chunks, tile_chunk_end, chunks_thru_span = _schedule(cfg, prep)

    n_chunks = len(chunks)
    n_calls = len(calls)
    n_spans = cfg.n_spans
    max_span_cols = max(span_cols)
    max_i_cols = max(sum(unit_cols[(s, i, b)] for b in range(NB))
                     for s in range(n_spans) for i in range(NS))

    pair_idx = {(s, i): s * NS + i for s in range(n_spans) for i in range(NS)}
    i_col0 = {(s, i): int(col_of[i, s * GSPAN, 0])
              for s in range(n_spans) for i in range(NS)}
    out_rows = {t: max(0, min(DT, cfg.npc - t * DT)) for t in range(NT)}
    n_valid_tiles = sum(1 for t in range(NT) if out_rows[t] > 0)

    msg_dt = mybir.dt.bfloat16

    nc = bacc.Bacc("TRN2", target_bir_lowering=False, debug=False,
                   num_devices=N_CORES, num_swdge_queues=4)
    x = nc.dram_tensor("x", [cfg.x_rows_pad, P], msg_dt, kind="ExternalInput").ap()
    idx_d = nc.dram_tensor("idx", [P, totc * 8], mybir.dt.int16,
                           kind="ExternalInput").ap()
    dstf_d = nc.dram_tensor("dstf", [P, totc], mybir.dt.float32,
                            kind="ExternalInput").ap()
    iota_d = nc.dram_tensor("iota", [P, DT], msg_dt, kind="ExternalInput").ap()
    w_d = nc.dram_tensor("w", [NS * P, P], mybir.dt.float32,
                         kind="ExternalInput").ap()
    out_d = nc.dram_tensor("out", [cfg.npc, P], mybir.dt.float32,
                           kind="ExternalOutput").ap()

    with (
        nc.Block() as block,
        nc.sbuf_tensor("iota_sb", [P, DT], msg_dt) as iota_sb,
        nc.sbuf_tensor("w_sb", [P, NS, P], mybir.dt.float32) as w_sb,
        nc.sbuf_tensor("idxb", [P, 2, max_span_cols * 8], mybir.dt.int16) as idxb,
        nc.sbuf_tensor("dstfb", [P, 2, max_span_cols], mybir.dt.float32) as dstfb,
        nc.sbuf_tensor("stg", [P, STGRING, max_i_cols, P], msg_dt) as stg,
        nc.sbuf_tensor("sring", [P, SRING, DT], msg_dt) as sring,
        nc.sbuf_tensor("gsb", [P, 2, NS, DT], mybir.dt.float32) as gsb,
        nc.sbuf_tensor("ot", [P, OTSLOTS, P], mybir.dt.float32) as ot,
        nc.psum_tensor("gp", [P, GPSLOTS, NS, DT], mybir.dt.float32) as gp,
        nc.psum_tensor("op", [P, OTSLOTS, P], mybir.dt.float32) as op,
        ExitStack() as _stack,
    ):
        _sem = lambda n: _stack.enter_context(nc.semaphore(n))
        cio = _sem("cio")    # const loads done (SP)
        sin = _sem("sin")    # span idx+dstf DMA done (SP, +32/span)
        sgr = [_sem(f"sgr{j}") for j in range(NGSEM)]  # per-call rotating
        sstg = _sem("sstg")  # stg (s,i) consumed by PE (+1)
        ss = _sem("ss")      # one-hot ready (DVE, +1/chunk)
        ssf = _sem("ssf")    # one-hot consumed (PE, +1/chunk)
        st = _sem("st")      # tile psum complete (PE, +1/tile)
        sgp = _sem("sgp")    # gsb ready / gpsum freed (ACT, +1/tile)
        sp2 = _sem("sp2")    # opsum ready (PE, +1/tile)
        sot = _sem("sot")    # ot ready (ACT, +1/tile)
        sof = _sem("sof")    # out DMA done (SP, +16/tile)

        @block.sync
        def _(sy: bass.BassEngine):
            sy.dma_start(iota_sb[:], iota_d[:]).then_inc(cio, 16)
            for i in range(NS):
                sy.dma_start(w_sb[:, i, :],
                             w_d[i * P:(i + 1) * P, :]).then_inc(cio, 16)
            tile_seq = 0
            od = 0  # completed-out-DMA self-wait counter

            def emit_out(t):
                nonlocal tile_seq, od
                rows = out_rows[t]
                if rows <= 0:
                    tile_seq += 1
                    return
                sy.wait_ge(sot, tile_seq + 1)
                if od > 0:
                    sy.wait_ge(sof, 16 * od)  # updater-order: prior outs done
                sy.dma_start(out_d[t * DT:t * DT + rows, :],
                             ot[:rows, tile_seq % OTSLOTS, :]).then_inc(sof, 16)
                od += 1
                tile_seq += 1

            for rep in range(reps):
                for s in range(n_spans):
                    gs = rep * n_spans + s
                    if gs >= 1:
                        sy.wait_ge(sin, 32 * gs)  # updater-order: prior ins done
                    if gs >= 2:
                        ps = gs - 2
                        for j, rnd in sorted(span_waits[ps % n_spans].items()):
                            sy.wait_ge(sgr[j], 16 * ((ps // n_spans)
                                                     * slot_tot[j] + rnd))
                        sy.wait_ge(ss, (ps // n_spans) * n_chunks
                                   + chunks_thru_span[ps % n_spans])
                    c0, cw = span_col0[s], span_cols[s]
                    sy.dma_start(idxb[:, gs % 2, 0:cw * 8],
                                 idx_d[:, c0 * 8:(c0 + cw) * 8]).then_inc(sin, 16)
                    sy.wait_ge(sin, 32 * gs + 16)
                    sy.dma_start(dstfb[:, gs % 2, 0:cw],
                                 dstf_d[:, c0:c0 + cw]).then_inc(sin, 16)
                    if gs >= 2:
                        ps_s = (gs - 2) % n_spans
                        for t in range(ps_s * GSPAN,
                                       min((ps_s + 1) * GSPAN, NT)):
                            emit_out(t)
            for gs_tr in range(max(0, reps * n_spans - 2), reps * n_spans):
                s = gs_tr % n_spans
                for t in range(s * GSPAN, min((s + 1) * GSPAN, NT)):
                    emit_out(t)
            for j in range(NGSEM):
                if slot_tot[j] > 0:
                    sy.wait_ge(sgr[j], 16 * reps * slot_tot[j])
            sy.wait_ge(sof, 16 * n_valid_tiles * reps)

        @block.gpsimd
        def _(g: bass.BassGpSimd):
            g.load_library(mlp)
            g.wait_ge(cio, 16 * (1 + NS))
            ci = 0
            cur_span = -1
            waited_pair = -1
            for rep in range(reps):
                for cl in calls:
                    s, i, b = cl["s"], cl["i"], cl["b"]
                    gs = rep * n_spans + s
                    gpair = rep * n_spans * NS + pair_idx[(s, i)]
                    if gs != cur_span:
                        g.wait_ge(sin, 32 * (gs + 1))
                        cur_span = gs
                    if gpair >= STGRING and gpair != waited_pair:
                        g.wait_ge(sstg, gpair - STGRING + 1)
                        waited_pair = gpair
                    k_s = rep * slot_tot[cl["slot"]] + cl["round"]
                    if k_s > 1:
                        g.wait_ge(sgr[cl["slot"]], 16 * (k_s - 1))
                    rel = cl["col0"] - i_col0[(s, i)]
                    srel = cl["col0"] - span_col0[s]
                    n_idx = cl["cols"] * P
                    g.dma_gather(
                        out_ap=stg[:, gpair % STGRING, rel:rel + cl["cols"], :],
                        in_ap=x[b * BANK_ROWS:(b + 1) * BANK_ROWS, :],
                        idxs_ap=idxb[:, gs % 2, srel * 8:(srel + cl["cols"]) * 8],
                        num_idxs=n_idx,
                        num_idxs_reg=n_idx,
                        elem_size=P,
                        single_packet=False,
                        queue_num=cl["q"],
                    ).then_inc(sgr[cl["slot"]], 16)
                    ci += 1

        @block.vector
        def _(v: bass.BassVectorEngine):
            v.wait_ge(cio, 16 * (1 + NS))
            idx = 0
            cur_span = -1
            for rep in range(reps):
                for ch in chunks:
                    gs = rep * n_spans + ch["s"]
                    if gs != cur_span:
                        v.wait_ge(sin, 32 * (gs + 1))
                        cur_span = gs
                    if idx >= SRING and idx % 4 == 0:
                        v.wait_ge(ssf, idx + 4 - SRING)
                    scol = ch["col"] - span_col0[ch["s"]]
                    v.tensor_scalar(
                        out=sring[:, idx % SRING, :], in0=iota_sb[:],
                        scalar1=dstfb[:, gs % 2, scol:scol + 1],
                        scalar2=None, op0=mybir.AluOpType.is_equal,
                    ).then_inc(ss, 1)
                    idx += 1

        @block.tensor
        def _(t_: bass.BassTensorEngine):
            t_.wait_ge(cio, 16 * (1 + NS))
            idx = 0
            tile_seq = 0
            pend = []  # tile_seqs awaiting phase2 (depth 2 for ACT slack)

            def phase2(tseq):
                t_.wait_ge(sgp, tseq + 1)
                if tseq + 1 > OTSLOTS:
                    t_.wait_ge(sot, tseq + 1 - OTSLOTS)
                for i in range(NS):
                    mm = t_.matmul(
                        out=op[:, tseq % OTSLOTS, :],
                        lhsT=gsb[:, tseq % 2, i, :],
                        rhs=w_sb[:, i, :],
                        start=(i == 0), stop=(i == NS - 1))
                    if i == NS - 1:
                        mm.then_inc(sp2, 1)

            for rep in range(reps):
                last_ti = None
                for ch in chunks:
                    s, t, i, b = ch["s"], ch["t"], ch["i"], ch["b"]
                    # first chunk of tile t overall (i == 0 pass)
                    if i == 0 and (t, rep) != last_ti and ch["start"]:
                        if len(pend) >= 2:
                            phase2(pend.pop(0))
                        ts_new = rep * NT + t
                        if ts_new >= GPSLOTS:
                            t_.wait_ge(sgp, ts_new - GPSLOTS + 1)
                        last_ti = (t, rep)
                    if ch["first_of_group"]:
                        for j, rnd in group_waits[(s, i, b)]:
                            t_.wait_ge(sgr[j], 16 * (rep * slot_tot[j] + rnd))
                    t_.wait_ge(ss, idx + 1)
                    gpair = rep * n_spans * NS + pair_idx[(s, i)]
                    rel = ch["col"] - i_col0[(s, i)]
                    ts_cur = rep * NT + t
                    mm = t_.matmul(
                        out=gp[:, ts_cur % GPSLOTS, i, :],
                        lhsT=stg[:, gpair % STGRING, rel, :],
                        rhs=sring[:, idx % SRING, :],
                        start=ch["start"], stop=ch["stop"],
                    )
                    mm.then_inc(ssf, 1)
                    idx += 1
                    if ch["last_of_pair"]:
                        t_.drain().then_inc(sstg, 1)
                    if idx - rep * n_chunks == tile_chunk_end[t]:
                        t_.drain().then_inc(st, 1)
                        pend.append(tile_seq)
                        tile_seq += 1
                while pend:
                    phase2(pend.pop(0))

        @block.scalar
        def _(a: bass.BassScalarEngine):
            tile_seq = 0
            for rep in range(reps):
                for t in range(NT):
                    a.wait_ge(st, tile_seq + 1)
                    if tile_seq >= 2:
                        a.wait_ge(sp2, tile_seq - 1)
                    a.activation(out=gsb[:, tile_seq % 2, :, :],
                                 in_=gp[:, tile_seq % GPSLOTS, :, :],
                                 func=mybir.ActivationFunctionType.Copy
                                 ).then_inc(sgp, 1)
                    a.wait_ge(sp2, tile_seq + 1)
                    if tile_seq + 1 > OTSLOTS:
                        a.wait_ge(sof, 16 * (tile_seq + 1 - OTSLOTS))
                    a.activation(out=ot[:, tile_seq % OTSLOTS, :],
                                 in_=op[:, tile_seq % OTSLOTS, :],
                                 func=mybir.ActivationFunctionType.Copy
                                 ).then_inc(sot, 1)
                    tile_seq += 1

    nc.compile()
    return nc


def _make_in_maps(cfg, prep, x, w_list):
    n_nodes = x.shape[0]
    x_pad = np.zeros((cfg.x_rows_pad, P), np.float32)
    x_pad[:n_nodes] = np.asarray(x, np.float32)
    x_pad = x_pad.astype(ml_dtypes.bfloat16)
    iota = np.tile(np.arange(DT, dtype=np.float32)[None, :], (P, 1)).astype(
        ml_dtypes.bfloat16)
    w_cat = np.concatenate([np.asarray(w, np.float32) for w in w_list], axis=0)

    return [{
        "x": x_pad,
        "idx": prep["idx_mats"][c],
        "dstf": prep["dstf_mats"][c],
        "iota": iota,
        "w": w_cat,
    } for c in range(N_CORES)]


def kernel(hidden_states, edges_i, edges_ii, edges_iii, edges_a,
           W_i, W_ii, W_iii, W_a):
    x = np.asarray(hidden_states, np.float32)
    cfg = _Cfg(x.shape[0])
    edges_list = [np.asarray(e) for e in (edges_i, edges_ii, edges_iii, edges_a)]
    w_list = [W_i, W_ii, W_iii, W_a]

    prep = _host_prep(cfg, edges_list)
    nc = _build_kernel(cfg, prep)
    in_maps = _make_in_maps(cfg, prep, x, w_list)

    res = run_bass_kernel_spmd(nc, in_maps, core_ids=list(range(N_CORES)))
    out = np.concatenate([res.results[c]["out"] for c in range(N_CORES)], axis=0)
    return out.astype(np.float32)


# revision 67
# speedup vs baseline: 1.1963x; 1.1963x over previous
"""GCN message-passing layer (4x GCNConv sum) on 8 Trainium2 NeuronCores.

out[d] = sum_i ( segment_sum_{e in E_i, dst=d} x[src_e] ) @ W_i

Raw-block SPMD kernel (no Tile scheduler): destination nodes sharded across
8 cores, x replicated. Per core:
  - Host groups edges by (core, set, dst-tile(128), src-bank(32768)), sorts
    each group by src (DRAM locality), pads groups to a cross-core-uniform
    multiple of 128 (pad: src=0, dstf=-1).
  - gpsimd dma_gather (int16 bank-local idxs) stages x[src] rows (bf16) into
    SBUF, round-robin over 4 SWDGE queues (4x descriptor throughput).
  - Processing is set-major within each 4-tile span: while PE consumes set
    i's staging, gpsimd prefetches sets i+1, i+2 (stg ring of 3).
  - DVE builds one-hot S[e,d] = (dstf[e]==d) per 128-edge chunk; TensorE
    accumulates g_i^T = stg^T @ S into per-tile PSUM (1 bank, 6-slot ring).
  - ACT copies tile PSUM->SBUF; TensorE phase 2: out_tile = sum_i gsb_i^T.T
    @ W_i into opsum; ACT copies opsum->SBUF; SP DMAs out rows.
All cross-engine sync is explicit semaphores; every core runs an identical
program (counts are cross-core uniform by construction).
"""
import math
import sys
from contextlib import ExitStack

sys.path.insert(0, "/opt/trn_rl_repo")

import numpy as np
import ml_dtypes

from concourse import bass, mybir, bacc
from concourse.bass_utils import run_bass_kernel_spmd
from concourse.library_config import mlp

P = 128
N_CORES = 8
DT = 128           # dst-tile width (one PSUM bank per (tile, 4 sets))
BANK_ROWS = 32768
NS = 4
GSPAN = 3          # tiles per idx/gather span (must be <= GPSLOTS)
CALL_COLS = 112    # max 128-edge chunks per dma_gather call (14336 idxs)
SRING = 16         # one-hot ring slots
STGRING = 8        # stg (span,bank) block ring slots (>= 2*NB)
GPSLOTS = 6        # gpsum PSUM rotation (1 bank each)
OTSLOTS = 4        # opsum/ot rotation


class _Cfg:
    def __init__(self, n_nodes):
        self.n_nodes = n_nodes
        self.npc = n_nodes // N_CORES
        self.nt = math.ceil(self.npc / DT)
        self.n_banks = math.ceil(n_nodes / BANK_ROWS)
        self.x_rows_pad = self.n_banks * BANK_ROWS
        self.n_spans = math.ceil(self.nt / GSPAN)


def _host_prep(cfg, edges_list):
    NC, NT, NB = N_CORES, cfg.nt, cfg.n_banks
    counts = np.zeros((NC, NS, NT, NB), np.int64)
    per_set = []
    for i, e in enumerate(edges_list):
        src = np.asarray(e[0], np.int64)
        dst = np.asarray(e[1], np.int64)
        core = dst // cfg.npc
        dloc = dst % cfg.npc
        t = dloc // DT
        b = src // BANK_ROWS
        key = (core * NT + t) * NB + b
        counts[:, i] = np.bincount(key, minlength=NC * NT * NB).reshape(NC, NT, NB)
        order = np.lexsort((src, key))  # src-sorted within group: DRAM locality
        per_set.append((src[order], dloc[order], key[order]))

    C = -(-counts.max(axis=0) // P)
    # guarantee no (set, tile) is entirely empty (phase-2 reads its psum)
    for i in range(NS):
        for t in range(NT):
            if C[i, t].sum() == 0:
                C[i, t, 0] = 1

    col_of = np.zeros((NS, NT, NB), np.int64)
    blk_col0 = {}   # (s, b) -> first col of the bank block
    blk_cols = {}   # (s, b) -> cols in the block (all sets, span tiles)
    span_col0, span_cols = [], []
    col = 0
    for s in range(cfg.n_spans):
        ts = range(s * GSPAN, min((s + 1) * GSPAN, NT))
        span_col0.append(col)
        for b in range(NB):
            blk_col0[(s, b)] = col
            for i in range(NS):
                for t in ts:
                    col_of[i, t, b] = col
                    col += C[i, t, b]
            blk_cols[(s, b)] = col - blk_col0[(s, b)]
        span_cols.append(col - span_col0[-1])
    totc = col
    tot_slots = totc * P

    idx_mats, dstf_mats = [], []
    for c in range(NC):
        slot_src = np.zeros(tot_slots, np.int64)
        slot_dstf = np.full(tot_slots, -1.0, np.float32)
        for i in range(NS):
            src_s, dloc_s, key_s = per_set[i]
            lo = np.searchsorted(key_s, c * NT * NB)
            hi = np.searchsorted(key_s, (c + 1) * NT * NB)
            src_c, dloc_c, key_c = src_s[lo:hi], dloc_s[lo:hi], key_s[lo:hi]
            t_c = (key_c // NB) % NT
            b_c = key_c % NB
            gstart = np.searchsorted(key_c, key_c)
            rank = np.arange(len(key_c)) - gstart
            slot = col_of[i, t_c, b_c] * P + rank
            slot_src[slot] = src_c - b_c * BANK_ROWS
            slot_dstf[slot] = (dloc_c - t_c * DT).astype(np.float32)
        idx16 = slot_src.reshape(tot_slots // 16, 16).T.astype(np.int16)
        idx_mats.append(np.tile(idx16, (8, 1)))
        dstf_mats.append(slot_dstf.reshape(totc, P).T.copy())

    # gather call list: per (s,b) bank block, split into <=CALL_COLS chunks
    calls = []
    for s in range(cfg.n_spans):
        for b in range(NB):
            cols = blk_cols[(s, b)]
            if cols == 0:
                continue
            g0 = blk_col0[(s, b)]
            for c0 in range(0, cols, CALL_COLS):
                cw = min(CALL_COLS, cols - c0)
                calls.append(dict(s=s, b=b, col0=g0 + c0, cols=cw))
    # queue = ci % 4 (engine parallelism); completion sem = rotating ring of
    # NGSEM per-call sems (exact completion tracking: cumulative multi-queue
    # counts are unsound because the 16 SDMA engines drain independently).
    NGSEM = 16
    slot_rounds = [0] * NGSEM
    for ci, cl in enumerate(calls):
        cl["q"] = ci % 4
        slot = ci % NGSEM
        slot_rounds[slot] += 1
        cl["slot"] = slot
        cl["round"] = slot_rounds[slot]  # 1-based within rep
    group_waits = {}    # (s,b) -> [(slot, round)] for its calls
    span_waits = {}     # s -> {slot: max round} through end of span
    run = {}
    for cl in calls:
        g = (cl["s"], cl["b"])
        group_waits.setdefault(g, []).append((cl["slot"], cl["round"]))
        run[cl["slot"]] = cl["round"]
        span_waits[cl["s"]] = dict(run)

    return dict(C=C, col_of=col_of, totc=totc,
                blk_col0=blk_col0, blk_cols=blk_cols,
                span_col0=span_col0, span_cols=span_cols,
                idx_mats=idx_mats, dstf_mats=dstf_mats,
                calls=calls, group_waits=group_waits,
                span_waits=span_waits, slot_tot=slot_rounds, ngsem=NGSEM)


def _schedule(cfg, prep):
    """Static per-core schedule: ordered chunk list (set-major within span)
    + per-tile completion chunk counts."""
    C, col_of = prep["C"], prep["col_of"]
    NT, NB = cfg.nt, cfg.n_banks
    chunks = []
    tile_chunk_end = {}   # tile t -> chunk count (1-based) at its completion
    chunks_thru_span = {}
    for s in range(cfg.n_spans):
        ts = range(s * GSPAN, min((s + 1) * GSPAN, NT))
        for i in range(NS):
            for t in ts:
                nib = [(b, int(C[i, t, b])) for b in range(NB) if C[i, t, b] > 0]
                tot_k = sum(n for _, n in nib)
                kk = 0
                for b, cc in nib:
                    for k in range(cc):
                        chunks.append(dict(
                            s=s, t=t, i=i, b=b,
                            col=int(col_of[i, t, b]) + k,
                            start=(kk == 0), stop=(kk == tot_k - 1),
                            first_of_group=(k == 0),
                        ))
                        kk += 1
                if i == NS - 1:
                    tile_chunk_end[t] = len(chunks)
        chunks_thru_span[s] = len(chunks)
    return chunks, tile_chunk_end, chunks_thru_span


def _build_kernel(cfg, prep, reps=1, mode="full"):
    # mode: "full" | "gather" (gathers+input DMAs only, for profiling)
    gather_only = (mode == "gather")
    NT, NB = cfg.nt, cfg.n_banks
    calls = prep["calls"]
    group_waits = prep["group_waits"]
    span_waits = prep["span_waits"]
    slot_tot = prep["slot_tot"]
    NGSEM = prep["ngsem"]
    span_col0, span_cols = prep["span_col0"], prep["span_cols"]
    blk_col0, blk_cols = prep["blk_col0"], prep["blk_cols"]
    col_of = prep["col_of"]
    totc = prep["totc"]
    chunks, tile_chunk_end, chunks_thru_span = _schedule(cfg, prep)

    n_chunks = len(chunks)
    n_calls = len(calls)
    n_spans = cfg.n_spans
    max_span_cols = max(span_cols)
    max_blk_cols = max(blk_cols.values())

    pair_idx = {(s, b): s * NB + b for s in range(n_spans) for b in range(NB)}
    out_rows = {t: max(0, min(DT, cfg.npc - t * DT)) for t in range(NT)}
    n_valid_tiles = sum(1 for t in range(NT) if out_rows[t] > 0)

    msg_dt = mybir.dt.bfloat16

    nc = bacc.Bacc("TRN2", target_bir_lowering=False, debug=False,
                   num_devices=N_CORES, num_swdge_queues=4)
    x = nc.dram_tensor("x", [cfg.x_rows_pad, P], msg_dt, kind="ExternalInput").ap()
    idx_d = nc.dram_tensor("idx", [P, totc * 8], mybir.dt.int16,
                           kind="ExternalInput").ap()
    dstf_d = nc.dram_tensor("dstf", [P, totc], mybir.dt.float32,
                            kind="ExternalInput").ap()
    iota_d = nc.dram_tensor("iota", [P, DT], msg_dt, kind="ExternalInput").ap()
    w_d = nc.dram_tensor("w", [NS * P, P], mybir.dt.float32,
                         kind="ExternalInput").ap()
    out_d = nc.dram_tensor("out", [cfg.npc, P], mybir.dt.float32,
                           kind="ExternalOutput").ap()

    with (
        nc.Block() as block,
        nc.sbuf_tensor("iota_sb", [P, DT], msg_dt) as iota_sb,
        nc.sbuf_tensor("w_sb", [P, NS, P], mybir.dt.float32) as w_sb,
        nc.sbuf_tensor("idxb", [P, 2, max_span_cols * 8], mybir.dt.int16) as idxb,
        nc.sbuf_tensor("dstfb", [P, 2, max_span_cols], mybir.dt.float32) as dstfb,
        nc.sbuf_tensor("stg", [P, STGRING, max_blk_cols, P], msg_dt) as stg,
        nc.sbuf_tensor("sring", [P, SRING, DT], msg_dt) as sring,
        nc.sbuf_tensor("gsb", [P, 2, NS, DT], mybir.dt.float32) as gsb,
        nc.sbuf_tensor("ot", [P, OTSLOTS, P], mybir.dt.float32) as ot,
        nc.psum_tensor("gp", [P, GPSLOTS, NS, DT], mybir.dt.float32) as gp,
        nc.psum_tensor("op", [P, OTSLOTS, P], mybir.dt.float32) as op,
        ExitStack() as _stack,
    ):
        _sem = lambda n: _stack.enter_context(nc.semaphore(n))
        cio = _sem("cio")    # const loads done (SP)
        sin = _sem("sin")    # span idx+dstf DMA done (SP, +32/span)
        sgr = [_sem(f"sgr{j}") for j in range(NGSEM)]  # per-call rotating
        sstg = _sem("sstg")  # stg bank blocks consumed by PE (+NB/span)
        ss = _sem("ss")      # one-hot ready (DVE, +1/chunk)
        ssf = _sem("ssf")    # one-hot consumed (PE, +1/chunk)
        st = _sem("st")      # tile psum complete (PE, +1/tile)
        sgp = _sem("sgp")    # gsb ready / gpsum freed (ACT, +1/tile)
        sp2 = _sem("sp2")    # opsum ready (PE, +1/tile)
        sot = _sem("sot")    # ot ready (ACT, +1/tile)
        sof = _sem("sof")    # out DMA done (SP, +16/tile)

        @block.sync
        def _(sy: bass.BassEngine):
            sy.dma_start(iota_sb[:], iota_d[:]).then_inc(cio, 16)
            for i in range(NS):
                sy.dma_start(w_sb[:, i, :],
                             w_d[i * P:(i + 1) * P, :]).then_inc(cio, 16)
            tile_seq = 0
            od = 0  # completed-out-DMA self-wait counter

            def emit_out(t):
                nonlocal tile_seq, od
                rows = out_rows[t]
                if rows <= 0:
                    tile_seq += 1
                    return
                sy.wait_ge(sot, tile_seq + 1)
                if od > 0:
                    sy.wait_ge(sof, 16 * od)  # updater-order: prior outs done
                sy.dma_start(out_d[t * DT:t * DT + rows, :],
                             ot[:rows, tile_seq % OTSLOTS, :]).then_inc(sof, 16)
                od += 1
                tile_seq += 1

            for rep in range(reps):
                for s in range(n_spans):
                    gs = rep * n_spans + s
                    if gs >= 1:
                        sy.wait_ge(sin, 32 * gs)  # updater-order: prior ins done
                    if gs >= 2:
                        ps = gs - 2
                        for j, rnd in sorted(span_waits[ps % n_spans].items()):
                            sy.wait_ge(sgr[j], 16 * ((ps // n_spans)
                                                     * slot_tot[j] + rnd))
                        if not gather_only:
                            sy.wait_ge(ss, (ps // n_spans) * n_chunks
                                       + chunks_thru_span[ps % n_spans])
                    c0, cw = span_col0[s], span_cols[s]
                    sy.dma_start(idxb[:, gs % 2, 0:cw * 8],
                                 idx_d[:, c0 * 8:(c0 + cw) * 8]).then_inc(sin, 16)
                    sy.wait_ge(sin, 32 * gs + 16)
                    sy.dma_start(dstfb[:, gs % 2, 0:cw],
                                 dstf_d[:, c0:c0 + cw]).then_inc(sin, 16)
                    if gs >= 2 and not gather_only:
                        ps_s = (gs - 2) % n_spans
                        for t in range(ps_s * GSPAN,
                                       min((ps_s + 1) * GSPAN, NT)):
                            emit_out(t)
            for gs_tr in range(max(0, reps * n_spans - 2), reps * n_spans):
                if gather_only:
                    break
                s = gs_tr % n_spans
                for t in range(s * GSPAN, min((s + 1) * GSPAN, NT)):
                    emit_out(t)
            for j in range(NGSEM):
                if slot_tot[j] > 0:
                    sy.wait_ge(sgr[j], 16 * reps * slot_tot[j])
            if not gather_only:
                sy.wait_ge(sof, 16 * n_valid_tiles * reps)

        @block.gpsimd
        def _(g: bass.BassGpSimd):
            g.load_library(mlp)
            g.wait_ge(cio, 16 * (1 + NS))
            cur_span = -1
            for rep in range(reps):
                for cl in calls:
                    s, b = cl["s"], cl["b"]
                    gs = rep * n_spans + s
                    gpair = rep * n_spans * NB + pair_idx[(s, b)]
                    if gs != cur_span:
                        g.wait_ge(sin, 32 * (gs + 1))
                        if gs >= 2 and not gather_only:
                            # all NB blocks of span gs-2 released together
                            g.wait_ge(sstg, NB * (gs - 1))
                        cur_span = gs
                    k_s = rep * slot_tot[cl["slot"]] + cl["round"]
                    if k_s > 1:
                        g.wait_ge(sgr[cl["slot"]], 16 * (k_s - 1))
                    rel = cl["col0"] - blk_col0[(s, b)]
                    srel = cl["col0"] - span_col0[s]
                    n_idx = cl["cols"] * P
                    g.dma_gather(
                        out_ap=stg[:, gpair % STGRING, rel:rel + cl["cols"], :],
                        in_ap=x[b * BANK_ROWS:(b + 1) * BANK_ROWS, :],
                        idxs_ap=idxb[:, gs % 2, srel * 8:(srel + cl["cols"]) * 8],
                        num_idxs=n_idx,
                        num_idxs_reg=n_idx,
                        elem_size=P,
                        single_packet=False,
                        queue_num=cl["q"],
                    ).then_inc(sgr[cl["slot"]], 16)

        @block.vector
        def _(v: bass.BassVectorEngine):
            if gather_only:
                return
            v.wait_ge(cio, 16 * (1 + NS))
            idx = 0
            cur_span = -1
            for rep in range(reps):
                for ch in chunks:
                    gs = rep * n_spans + ch["s"]
                    if gs != cur_span:
                        v.wait_ge(sin, 32 * (gs + 1))
                        cur_span = gs
                    if idx >= SRING and idx % 4 == 0:
                        v.wait_ge(ssf, idx + 4 - SRING)
                    scol = ch["col"] - span_col0[ch["s"]]
                    v.tensor_scalar(
                        out=sring[:, idx % SRING, :], in0=iota_sb[:],
                        scalar1=dstfb[:, gs % 2, scol:scol + 1],
                        scalar2=None, op0=mybir.AluOpType.is_equal,
                    ).then_inc(ss, 1)
                    idx += 1

        @block.tensor
        def _(t_: bass.BassTensorEngine):
            if gather_only:
                return
            t_.wait_ge(cio, 16 * (1 + NS))
            idx = 0
            tile_seq = 0
            pend = []  # tile_seqs awaiting phase2 (depth 2 for ACT slack)

            def phase2(tseq):
                t_.wait_ge(sgp, tseq + 1)
                if tseq + 1 > OTSLOTS:
                    t_.wait_ge(sot, tseq + 1 - OTSLOTS)
                for i in range(NS):
                    mm = t_.matmul(
                        out=op[:, tseq % OTSLOTS, :],
                        lhsT=gsb[:, tseq % 2, i, :],
                        rhs=w_sb[:, i, :],
                        start=(i == 0), stop=(i == NS - 1))
                    if i == NS - 1:
                        mm.then_inc(sp2, 1)

            for rep in range(reps):
                last_ti = None
                for ch in chunks:
                    s, t, i, b = ch["s"], ch["t"], ch["i"], ch["b"]
                    # first chunk of tile t overall (i == 0 pass)
                    if i == 0 and (t, rep) != last_ti and ch["start"]:
                        if len(pend) >= 2:
                            phase2(pend.pop(0))
                        ts_new = rep * NT + t
                        if ts_new >= GPSLOTS:
                            t_.wait_ge(sgp, ts_new - GPSLOTS + 1)
                        last_ti = (t, rep)
                    if ch["first_of_group"]:
                        for j, rnd in group_waits[(s, b)]:
                            t_.wait_ge(sgr[j], 16 * (rep * slot_tot[j] + rnd))
                    t_.wait_ge(ss, idx + 1)
                    gpair = rep * n_spans * NB + pair_idx[(s, b)]
                    rel = ch["col"] - blk_col0[(s, b)]
                    ts_cur = rep * NT + t
                    mm = t_.matmul(
                        out=gp[:, ts_cur % GPSLOTS, i, :],
                        lhsT=stg[:, gpair % STGRING, rel, :],
                        rhs=sring[:, idx % SRING, :],
                        start=ch["start"], stop=ch["stop"],
                    )
                    mm.then_inc(ssf, 1)
                    idx += 1
                    if idx - rep * n_chunks == tile_chunk_end[t]:
                        t_.drain().then_inc(st, 1)
                        pend.append(tile_seq)
                        tile_seq += 1
                    if idx - rep * n_chunks == chunks_thru_span[s]:
                        # release all NB bank blocks of span s together
                        for _b in range(NB):
                            t_.drain().then_inc(sstg, 1)
                while pend:
                    phase2(pend.pop(0))

        @block.scalar
        def _(a: bass.BassScalarEngine):
            if gather_only:
                return
            tile_seq = 0
            for rep in range(reps):
                for t in range(NT):
                    a.wait_ge(st, tile_seq + 1)
                    if tile_seq >= 2:
                        a.wait_ge(sp2, tile_seq - 1)
                    a.activation(out=gsb[:, tile_seq % 2, :, :],
                                 in_=gp[:, tile_seq % GPSLOTS, :, :],
                                 func=mybir.ActivationFunctionType.Copy
                                 ).then_inc(sgp, 1)
                    a.wait_ge(sp2, tile_seq + 1)
                    if tile_seq + 1 > OTSLOTS:
                        a.wait_ge(sof, 16 * (tile_seq + 1 - OTSLOTS))
                    a.activation(out=ot[:, tile_seq % OTSLOTS, :],
                                 in_=op[:, tile_seq % OTSLOTS, :],
                                 func=mybir.ActivationFunctionType.Copy
                                 ).then_inc(sot, 1)
                    tile_seq += 1

    nc.compile()
    return nc


def _make_in_maps(cfg, prep, x, w_list):
    n_nodes = x.shape[0]
    x_pad = np.zeros((cfg.x_rows_pad, P), np.float32)
    x_pad[:n_nodes] = np.asarray(x, np.float32)
    x_pad = x_pad.astype(ml_dtypes.bfloat16)
    iota = np.tile(np.arange(DT, dtype=np.float32)[None, :], (P, 1)).astype(
        ml_dtypes.bfloat16)
    w_cat = np.concatenate([np.asarray(w, np.float32) for w in w_list], axis=0)

    return [{
        "x": x_pad,
        "idx": prep["idx_mats"][c],
        "dstf": prep["dstf_mats"][c],
        "iota": iota,
        "w": w_cat,
    } for c in range(N_CORES)]


def kernel(hidden_states, edges_i, edges_ii, edges_iii, edges_a,
           W_i, W_ii, W_iii, W_a):
    x = np.asarray(hidden_states, np.float32)
    cfg = _Cfg(x.shape[0])
    edges_list = [np.asarray(e) for e in (edges_i, edges_ii, edges_iii, edges_a)]
    w_list = [W_i, W_ii, W_iii, W_a]

    prep = _host_prep(cfg, edges_list)
    nc = _build_kernel(cfg, prep)
    in_maps = _make_in_maps(cfg, prep, x, w_list)

    res = run_bass_kernel_spmd(nc, in_maps, core_ids=list(range(N_CORES)))
    out = np.concatenate([res.results[c]["out"] for c in range(N_CORES)], axis=0)
    return out.astype(np.float32)


# revision 68
# speedup vs baseline: 1.2146x; 1.0153x over previous
"""GCN message-passing layer (4x GCNConv sum) on 8 Trainium2 NeuronCores.

out[d] = sum_i ( segment_sum_{e in E_i, dst=d} x[src_e] ) @ W_i

Raw-block SPMD kernel (no Tile scheduler): destination nodes sharded across
8 cores, x replicated. Per core:
  - Host groups edges by (core, set, dst-tile(128), src-bank(32768)), sorts
    each group by src (DRAM locality), pads groups to a cross-core-uniform
    multiple of 128 (pad: src=0, dstf=-1).
  - gpsimd dma_gather (int16 bank-local idxs) stages x[src] rows (bf16) into
    SBUF, round-robin over 4 SWDGE queues (4x descriptor throughput).
  - Processing is set-major within each 4-tile span: while PE consumes set
    i's staging, gpsimd prefetches sets i+1, i+2 (stg ring of 3).
  - DVE builds one-hot S[e,d] = (dstf[e]==d) per 128-edge chunk; TensorE
    accumulates g_i^T = stg^T @ S into per-tile PSUM (1 bank, 6-slot ring).
  - ACT copies tile PSUM->SBUF; TensorE phase 2: out_tile = sum_i gsb_i^T.T
    @ W_i into opsum; ACT copies opsum->SBUF; SP DMAs out rows.
All cross-engine sync is explicit semaphores; every core runs an identical
program (counts are cross-core uniform by construction).
"""
import math
import sys
from contextlib import ExitStack

sys.path.insert(0, "/opt/trn_rl_repo")

import numpy as np
import ml_dtypes

from concourse import bass, mybir, bacc
from concourse.bass_utils import run_bass_kernel_spmd
from concourse.library_config import mlp

P = 128
N_CORES = 8
DT = 128           # dst-tile width (one PSUM bank per (tile, 4 sets))
BANK_ROWS = 32768
NS = 4
GSPAN = 3          # tiles per idx/gather span (must be <= GPSLOTS)
CALL_COLS = 112    # max 128-edge chunks per dma_gather call (14336 idxs)
SRING = 16         # one-hot ring slots
STGRING = 8        # stg (span,bank) block ring slots (>= 2*NB)
GPSLOTS = 6        # gpsum PSUM rotation (1 bank each)
OTSLOTS = 4        # opsum/ot rotation


class _Cfg:
    def __init__(self, n_nodes):
        self.n_nodes = n_nodes
        self.npc = n_nodes // N_CORES
        self.nt = math.ceil(self.npc / DT)
        self.n_banks = math.ceil(n_nodes / BANK_ROWS)
        self.x_rows_pad = self.n_banks * BANK_ROWS
        self.n_spans = math.ceil(self.nt / GSPAN)


def _host_prep(cfg, edges_list):
    NC, NT, NB = N_CORES, cfg.nt, cfg.n_banks
    counts = np.zeros((NC, NS, NT, NB), np.int64)
    per_set = []
    for i, e in enumerate(edges_list):
        src = np.asarray(e[0], np.int64)
        dst = np.asarray(e[1], np.int64)
        core = dst // cfg.npc
        dloc = dst % cfg.npc
        t = dloc // DT
        b = src // BANK_ROWS
        key = (core * NT + t) * NB + b
        counts[:, i] = np.bincount(key, minlength=NC * NT * NB).reshape(NC, NT, NB)
        order = np.argsort(key, kind="stable")
        per_set.append((src[order], dloc[order], key[order]))

    C = -(-counts.max(axis=0) // P)
    # guarantee no (set, tile) is entirely empty (phase-2 reads its psum)
    for i in range(NS):
        for t in range(NT):
            if C[i, t].sum() == 0:
                C[i, t, 0] = 1

    col_of = np.zeros((NS, NT, NB), np.int64)
    blk_col0 = {}   # (s, b) -> first col of the bank block
    blk_cols = {}   # (s, b) -> cols in the block (all sets, span tiles)
    span_col0, span_cols = [], []
    col = 0
    for s in range(cfg.n_spans):
        ts = range(s * GSPAN, min((s + 1) * GSPAN, NT))
        span_col0.append(col)
        for b in range(NB):
            blk_col0[(s, b)] = col
            for i in range(NS):
                for t in ts:
                    col_of[i, t, b] = col
                    col += C[i, t, b]
            blk_cols[(s, b)] = col - blk_col0[(s, b)]
        span_cols.append(col - span_col0[-1])
    totc = col
    tot_slots = totc * P

    idx_mats, dstf_mats = [], []
    for c in range(NC):
        slot_src = np.zeros(tot_slots, np.int64)
        slot_dstf = np.full(tot_slots, -1.0, np.float32)
        for i in range(NS):
            src_s, dloc_s, key_s = per_set[i]
            lo = np.searchsorted(key_s, c * NT * NB)
            hi = np.searchsorted(key_s, (c + 1) * NT * NB)
            src_c, dloc_c, key_c = src_s[lo:hi], dloc_s[lo:hi], key_s[lo:hi]
            t_c = (key_c // NB) % NT
            b_c = key_c % NB
            gstart = np.searchsorted(key_c, key_c)
            rank = np.arange(len(key_c)) - gstart
            slot = col_of[i, t_c, b_c] * P + rank
            slot_src[slot] = src_c - b_c * BANK_ROWS
            slot_dstf[slot] = (dloc_c - t_c * DT).astype(np.float32)
        idx16 = slot_src.reshape(tot_slots // 16, 16).T.astype(np.int16)
        idx_mats.append(np.tile(idx16, (8, 1)))
        dstf_mats.append(slot_dstf.reshape(totc, P).T.copy())

    # gather call list: per (s,b) bank block, split into <=CALL_COLS chunks
    calls = []
    for s in range(cfg.n_spans):
        for b in range(NB):
            cols = blk_cols[(s, b)]
            if cols == 0:
                continue
            g0 = blk_col0[(s, b)]
            for c0 in range(0, cols, CALL_COLS):
                cw = min(CALL_COLS, cols - c0)
                calls.append(dict(s=s, b=b, col0=g0 + c0, cols=cw))
    # queue = ci % 4 (engine parallelism); completion sem = rotating ring of
    # NGSEM per-call sems (exact completion tracking: cumulative multi-queue
    # counts are unsound because the 16 SDMA engines drain independently).
    NGSEM = 16
    slot_rounds = [0] * NGSEM
    for ci, cl in enumerate(calls):
        cl["q"] = ci % 4
        slot = ci % NGSEM
        slot_rounds[slot] += 1
        cl["slot"] = slot
        cl["round"] = slot_rounds[slot]  # 1-based within rep
    group_waits = {}    # (s,b) -> [(slot, round)] for its calls
    span_waits = {}     # s -> {slot: max round} through end of span
    run = {}
    for cl in calls:
        g = (cl["s"], cl["b"])
        group_waits.setdefault(g, []).append((cl["slot"], cl["round"]))
        run[cl["slot"]] = cl["round"]
        span_waits[cl["s"]] = dict(run)

    return dict(C=C, col_of=col_of, totc=totc,
                blk_col0=blk_col0, blk_cols=blk_cols,
                span_col0=span_col0, span_cols=span_cols,
                idx_mats=idx_mats, dstf_mats=dstf_mats,
                calls=calls, group_waits=group_waits,
                span_waits=span_waits, slot_tot=slot_rounds, ngsem=NGSEM)


def _schedule(cfg, prep):
    """Static per-core schedule: ordered chunk list (set-major within span)
    + per-tile completion chunk counts."""
    C, col_of = prep["C"], prep["col_of"]
    NT, NB = cfg.nt, cfg.n_banks
    chunks = []
    tile_chunk_end = {}   # tile t -> chunk count (1-based) at its completion
    chunks_thru_span = {}
    for s in range(cfg.n_spans):
        ts = range(s * GSPAN, min((s + 1) * GSPAN, NT))
        for i in range(NS):
            for t in ts:
                nib = [(b, int(C[i, t, b])) for b in range(NB) if C[i, t, b] > 0]
                tot_k = sum(n for _, n in nib)
                kk = 0
                for b, cc in nib:
                    for k in range(cc):
                        chunks.append(dict(
                            s=s, t=t, i=i, b=b,
                            col=int(col_of[i, t, b]) + k,
                            start=(kk == 0), stop=(kk == tot_k - 1),
                            first_of_group=(k == 0),
                        ))
                        kk += 1
                if i == NS - 1:
                    tile_chunk_end[t] = len(chunks)
        chunks_thru_span[s] = len(chunks)
    return chunks, tile_chunk_end, chunks_thru_span


def _build_kernel(cfg, prep, reps=1, mode="full"):
    # mode: "full" | "gather" (gathers+input DMAs only, for profiling)
    gather_only = (mode == "gather")
    NT, NB = cfg.nt, cfg.n_banks
    calls = prep["calls"]
    group_waits = prep["group_waits"]
    span_waits = prep["span_waits"]
    slot_tot = prep["slot_tot"]
    NGSEM = prep["ngsem"]
    span_col0, span_cols = prep["span_col0"], prep["span_cols"]
    blk_col0, blk_cols = prep["blk_col0"], prep["blk_cols"]
    col_of = prep["col_of"]
    totc = prep["totc"]
    chunks, tile_chunk_end, chunks_thru_span = _schedule(cfg, prep)

    n_chunks = len(chunks)
    n_calls = len(calls)
    n_spans = cfg.n_spans
    max_span_cols = max(span_cols)
    max_blk_cols = max(blk_cols.values())

    pair_idx = {(s, b): s * NB + b for s in range(n_spans) for b in range(NB)}
    out_rows = {t: max(0, min(DT, cfg.npc - t * DT)) for t in range(NT)}
    n_valid_tiles = sum(1 for t in range(NT) if out_rows[t] > 0)

    msg_dt = mybir.dt.bfloat16

    nc = bacc.Bacc("TRN2", target_bir_lowering=False, debug=False,
                   num_devices=N_CORES, num_swdge_queues=4)
    x = nc.dram_tensor("x", [cfg.x_rows_pad, P], msg_dt, kind="ExternalInput").ap()
    idx_d = nc.dram_tensor("idx", [P, totc * 8], mybir.dt.int16,
                           kind="ExternalInput").ap()
    dstf_d = nc.dram_tensor("dstf", [P, totc], mybir.dt.float32,
                            kind="ExternalInput").ap()
    iota_d = nc.dram_tensor("iota", [P, DT], msg_dt, kind="ExternalInput").ap()
    w_d = nc.dram_tensor("w", [NS * P, P], mybir.dt.float32,
                         kind="ExternalInput").ap()
    out_d = nc.dram_tensor("out", [cfg.npc, P], mybir.dt.float32,
                           kind="ExternalOutput").ap()

    with (
        nc.Block() as block,
        nc.sbuf_tensor("iota_sb", [P, DT], msg_dt) as iota_sb,
        nc.sbuf_tensor("w_sb", [P, NS, P], mybir.dt.float32) as w_sb,
        nc.sbuf_tensor("idxb", [P, 2, max_span_cols * 8], mybir.dt.int16) as idxb,
        nc.sbuf_tensor("dstfb", [P, 2, max_span_cols], mybir.dt.float32) as dstfb,
        nc.sbuf_tensor("stg", [P, STGRING, max_blk_cols, P], msg_dt) as stg,
        nc.sbuf_tensor("sring", [P, SRING, DT], msg_dt) as sring,
        nc.sbuf_tensor("gsb", [P, 2, NS, DT], mybir.dt.float32) as gsb,
        nc.sbuf_tensor("ot", [P, OTSLOTS, P], mybir.dt.float32) as ot,
        nc.psum_tensor("gp", [P, GPSLOTS, NS, DT], mybir.dt.float32) as gp,
        nc.psum_tensor("op", [P, OTSLOTS, P], mybir.dt.float32) as op,
        ExitStack() as _stack,
    ):
        _sem = lambda n: _stack.enter_context(nc.semaphore(n))
        cio = _sem("cio")    # const loads done (SP)
        sin = _sem("sin")    # span idx+dstf DMA done (SP, +32/span)
        sgr = [_sem(f"sgr{j}") for j in range(NGSEM)]  # per-call rotating
        sstg = _sem("sstg")  # stg bank blocks consumed by PE (+NB/span)
        ss = _sem("ss")      # one-hot ready (DVE, +1/chunk)
        ssf = _sem("ssf")    # one-hot consumed (PE, +1/chunk)
        st = _sem("st")      # tile psum complete (PE, +1/tile)
        sgp = _sem("sgp")    # gsb ready / gpsum freed (ACT, +1/tile)
        sp2 = _sem("sp2")    # opsum ready (PE, +1/tile)
        sot = _sem("sot")    # ot ready (ACT, +1/tile)
        sof = _sem("sof")    # out DMA done (SP, +16/tile)

        @block.sync
        def _(sy: bass.BassEngine):
            sy.dma_start(iota_sb[:], iota_d[:]).then_inc(cio, 16)
            for i in range(NS):
                sy.dma_start(w_sb[:, i, :],
                             w_d[i * P:(i + 1) * P, :]).then_inc(cio, 16)
            tile_seq = 0
            od = 0  # completed-out-DMA self-wait counter

            def emit_out(t):
                nonlocal tile_seq, od
                rows = out_rows[t]
                if rows <= 0:
                    tile_seq += 1
                    return
                sy.wait_ge(sot, tile_seq + 1)
                if od > 0:
                    sy.wait_ge(sof, 16 * od)  # updater-order: prior outs done
                sy.dma_start(out_d[t * DT:t * DT + rows, :],
                             ot[:rows, tile_seq % OTSLOTS, :]).then_inc(sof, 16)
                od += 1
                tile_seq += 1

            for rep in range(reps):
                for s in range(n_spans):
                    gs = rep * n_spans + s
                    if gs >= 1:
                        sy.wait_ge(sin, 32 * gs)  # updater-order: prior ins done
                    if gs >= 2:
                        ps = gs - 2
                        for j, rnd in sorted(span_waits[ps % n_spans].items()):
                            sy.wait_ge(sgr[j], 16 * ((ps // n_spans)
                                                     * slot_tot[j] + rnd))
                        if not gather_only:
                            sy.wait_ge(ss, (ps // n_spans) * n_chunks
                                       + chunks_thru_span[ps % n_spans])
                    c0, cw = span_col0[s], span_cols[s]
                    sy.dma_start(idxb[:, gs % 2, 0:cw * 8],
                                 idx_d[:, c0 * 8:(c0 + cw) * 8]).then_inc(sin, 16)
                    sy.wait_ge(sin, 32 * gs + 16)
                    sy.dma_start(dstfb[:, gs % 2, 0:cw],
                                 dstf_d[:, c0:c0 + cw]).then_inc(sin, 16)
                    if gs >= 2 and not gather_only:
                        ps_s = (gs - 2) % n_spans
                        for t in range(ps_s * GSPAN,
                                       min((ps_s + 1) * GSPAN, NT)):
                            emit_out(t)
            for gs_tr in range(max(0, reps * n_spans - 2), reps * n_spans):
                if gather_only:
                    break
                s = gs_tr % n_spans
                for t in range(s * GSPAN, min((s + 1) * GSPAN, NT)):
                    emit_out(t)
            for j in range(NGSEM):
                if slot_tot[j] > 0:
                    sy.wait_ge(sgr[j], 16 * reps * slot_tot[j])
            if not gather_only:
                sy.wait_ge(sof, 16 * n_valid_tiles * reps)

        @block.gpsimd
        def _(g: bass.BassGpSimd):
            g.load_library(mlp)
            g.wait_ge(cio, 16 * (1 + NS))
            cur_span = -1
            for rep in range(reps):
                for cl in calls:
                    s, b = cl["s"], cl["b"]
                    gs = rep * n_spans + s
                    gpair = rep * n_spans * NB + pair_idx[(s, b)]
                    if gs != cur_span:
                        g.wait_ge(sin, 32 * (gs + 1))
                        if gs >= 2 and not gather_only:
                            # all NB blocks of span gs-2 released together
                            g.wait_ge(sstg, NB * (gs - 1))
                        cur_span = gs
                    k_s = rep * slot_tot[cl["slot"]] + cl["round"]
                    if k_s > 1:
                        g.wait_ge(sgr[cl["slot"]], 16 * (k_s - 1))
                    rel = cl["col0"] - blk_col0[(s, b)]
                    srel = cl["col0"] - span_col0[s]
                    n_idx = cl["cols"] * P
                    g.dma_gather(
                        out_ap=stg[:, gpair % STGRING, rel:rel + cl["cols"], :],
                        in_ap=x[b * BANK_ROWS:(b + 1) * BANK_ROWS, :],
                        idxs_ap=idxb[:, gs % 2, srel * 8:(srel + cl["cols"]) * 8],
                        num_idxs=n_idx,
                        num_idxs_reg=n_idx,
                        elem_size=P,
                        single_packet=False,
                        queue_num=cl["q"],
                    ).then_inc(sgr[cl["slot"]], 16)

        @block.vector
        def _(v: bass.BassVectorEngine):
            if gather_only:
                return
            v.wait_ge(cio, 16 * (1 + NS))
            idx = 0
            cur_span = -1
            for rep in range(reps):
                for ch in chunks:
                    gs = rep * n_spans + ch["s"]
                    if gs != cur_span:
                        v.wait_ge(sin, 32 * (gs + 1))
                        cur_span = gs
                    if idx >= SRING and idx % 4 == 0:
                        v.wait_ge(ssf, idx + 4 - SRING)
                    scol = ch["col"] - span_col0[ch["s"]]
                    v.tensor_scalar(
                        out=sring[:, idx % SRING, :], in0=iota_sb[:],
                        scalar1=dstfb[:, gs % 2, scol:scol + 1],
                        scalar2=None, op0=mybir.AluOpType.is_equal,
                    ).then_inc(ss, 1)
                    idx += 1

        @block.tensor
        def _(t_: bass.BassTensorEngine):
            if gather_only:
                return
            t_.wait_ge(cio, 16 * (1 + NS))
            idx = 0
            tile_seq = 0
            pend = []  # tile_seqs awaiting phase2 (depth 2 for ACT slack)

            def phase2(tseq):
                t_.wait_ge(sgp, tseq + 1)
                if tseq + 1 > OTSLOTS:
                    t_.wait_ge(sot, tseq + 1 - OTSLOTS)
                for i in range(NS):
                    mm = t_.matmul(
                        out=op[:, tseq % OTSLOTS, :],
                        lhsT=gsb[:, tseq % 2, i, :],
                        rhs=w_sb[:, i, :],
                        start=(i == 0), stop=(i == NS - 1))
                    if i == NS - 1:
                        mm.then_inc(sp2, 1)

            for rep in range(reps):
                last_ti = None
                for ch in chunks:
                    s, t, i, b = ch["s"], ch["t"], ch["i"], ch["b"]
                    # first chunk of tile t overall (i == 0 pass)
                    if i == 0 and (t, rep) != last_ti and ch["start"]:
                        if len(pend) >= 2:
                            phase2(pend.pop(0))
                        ts_new = rep * NT + t
                        if ts_new >= GPSLOTS:
                            t_.wait_ge(sgp, ts_new - GPSLOTS + 1)
                        last_ti = (t, rep)
                    if ch["first_of_group"]:
                        for j, rnd in group_waits[(s, b)]:
                            t_.wait_ge(sgr[j], 16 * (rep * slot_tot[j] + rnd))
                    t_.wait_ge(ss, idx + 1)
                    gpair = rep * n_spans * NB + pair_idx[(s, b)]
                    rel = ch["col"] - blk_col0[(s, b)]
                    ts_cur = rep * NT + t
                    mm = t_.matmul(
                        out=gp[:, ts_cur % GPSLOTS, i, :],
                        lhsT=stg[:, gpair % STGRING, rel, :],
                        rhs=sring[:, idx % SRING, :],
                        start=ch["start"], stop=ch["stop"],
                    )
                    mm.then_inc(ssf, 1)
                    idx += 1
                    if idx - rep * n_chunks == tile_chunk_end[t]:
                        t_.drain().then_inc(st, 1)
                        pend.append(tile_seq)
                        tile_seq += 1
                    if idx - rep * n_chunks == chunks_thru_span[s]:
                        # release all NB bank blocks of span s together
                        for _b in range(NB):
                            t_.drain().then_inc(sstg, 1)
                while pend:
                    phase2(pend.pop(0))

        @block.scalar
        def _(a: bass.BassScalarEngine):
            if gather_only:
                return
            tile_seq = 0
            for rep in range(reps):
                for t in range(NT):
                    a.wait_ge(st, tile_seq + 1)
                    if tile_seq >= 2:
                        a.wait_ge(sp2, tile_seq - 1)
                    a.activation(out=gsb[:, tile_seq % 2, :, :],
                                 in_=gp[:, tile_seq % GPSLOTS, :, :],
                                 func=mybir.ActivationFunctionType.Copy
                                 ).then_inc(sgp, 1)
                    a.wait_ge(sp2, tile_seq + 1)
                    if tile_seq + 1 > OTSLOTS:
                        a.wait_ge(sof, 16 * (tile_seq + 1 - OTSLOTS))
                    a.activation(out=ot[:, tile_seq % OTSLOTS, :],
                                 in_=op[:, tile_seq % OTSLOTS, :],
                                 func=mybir.ActivationFunctionType.Copy
                                 ).then_inc(sot, 1)
                    tile_seq += 1

    nc.compile()
    return nc


def _make_in_maps(cfg, prep, x, w_list):
    n_nodes = x.shape[0]
    x_pad = np.zeros((cfg.x_rows_pad, P), np.float32)
    x_pad[:n_nodes] = np.asarray(x, np.float32)
    x_pad = x_pad.astype(ml_dtypes.bfloat16)
    iota = np.tile(np.arange(DT, dtype=np.float32)[None, :], (P, 1)).astype(
        ml_dtypes.bfloat16)
    w_cat = np.concatenate([np.asarray(w, np.float32) for w in w_list], axis=0)

    return [{
        "x": x_pad,
        "idx": prep["idx_mats"][c],
        "dstf": prep["dstf_mats"][c],
        "iota": iota,
        "w": w_cat,
    } for c in range(N_CORES)]


def kernel(hidden_states, edges_i, edges_ii, edges_iii, edges_a,
           W_i, W_ii, W_iii, W_a):
    x = np.asarray(hidden_states, np.float32)
    cfg = _Cfg(x.shape[0])
    edges_list = [np.asarray(e) for e in (edges_i, edges_ii, edges_iii, edges_a)]
    w_list = [W_i, W_ii, W_iii, W_a]

    prep = _host_prep(cfg, edges_list)
    nc = _build_kernel(cfg, prep)
    in_maps = _make_in_maps(cfg, prep, x, w_list)

    res = run_bass_kernel_spmd(nc, in_maps, core_ids=list(range(N_CORES)))
    out = np.concatenate([res.results[c]["out"] for c in range(N_CORES)], axis=0)
    return out.astype(np.float32)


# revision 82
# speedup vs baseline: 1.2430x; 1.0235x over previous
"""GCN message-passing layer (4x GCNConv sum) on 8 Trainium2 NeuronCores.

out[d] = sum_i ( segment_sum_{e in E_i, dst=d} x[src_e] ) @ W_i

Raw-block SPMD kernel (no Tile scheduler): destination nodes sharded across
8 cores, x replicated. Per core:
  - Host groups edges by (core, set, dst-tile(128), src-bank(32768)), sorts
    each group by src (DRAM locality), pads groups to a cross-core-uniform
    multiple of 128 (pad: src=0, dstf=-1).
  - gpsimd dma_gather (int16 bank-local idxs) stages x[src] rows (bf16) into
    SBUF, round-robin over 4 SWDGE queues (4x descriptor throughput).
  - Processing is set-major within each 4-tile span: while PE consumes set
    i's staging, gpsimd prefetches sets i+1, i+2 (stg ring of 3).
  - DVE builds one-hot S[e,d] = (dstf[e]==d) per 128-edge chunk; TensorE
    accumulates g_i^T = stg^T @ S into per-tile PSUM (1 bank, 6-slot ring).
  - ACT copies tile PSUM->SBUF; TensorE phase 2: out_tile = sum_i gsb_i^T.T
    @ W_i into opsum; ACT copies opsum->SBUF; SP DMAs out rows.
All cross-engine sync is explicit semaphores; every core runs an identical
program (counts are cross-core uniform by construction).
"""
import math
import sys
from contextlib import ExitStack

sys.path.insert(0, "/opt/trn_rl_repo")

import numpy as np
import ml_dtypes

from concourse import bass, mybir, bacc
from concourse.bass_utils import run_bass_kernel_spmd
from concourse.library_config import mlp

P = 128
N_CORES = 8
DT = 256           # dst-tile width (two PSUM banks per (tile, 4 sets))
BANK_ROWS = 32768
NS = 4
GSPAN = 3          # tiles per idx/gather span (must be <= GPSLOTS)
CALL_COLS = 96     # max 128-edge chunks per dma_gather call (12288 idxs)
SRING = 8          # one-hot ring slots
STGRING = 5        # stg (span,bank) block ring slots (> NB)
GPSLOTS = 3        # gpsum PSUM rotation (2 banks each)
OTSLOTS = 2        # opsum/ot half-tile rotation (1 PSUM bank per slot)


class _Cfg:
    def __init__(self, n_nodes):
        self.n_nodes = n_nodes
        self.npc = n_nodes // N_CORES
        self.nt = math.ceil(self.npc / DT)
        self.n_banks = math.ceil(n_nodes / BANK_ROWS)
        self.x_rows_pad = self.n_banks * BANK_ROWS
        self.n_spans = math.ceil(self.nt / GSPAN)


def _host_prep(cfg, edges_list):
    NC, NT, NB = N_CORES, cfg.nt, cfg.n_banks
    counts = np.zeros((NC, NS, NT, NB), np.int64)
    per_set = []
    for i, e in enumerate(edges_list):
        src = np.asarray(e[0], np.int64)
        dst = np.asarray(e[1], np.int64)
        core = dst // cfg.npc
        dloc = dst % cfg.npc
        t = dloc // DT
        b = src // BANK_ROWS
        key = (core * NT + t) * NB + b
        counts[:, i] = np.bincount(key, minlength=NC * NT * NB).reshape(NC, NT, NB)
        order = np.argsort(key, kind="stable")
        per_set.append((src[order], dloc[order], key[order]))

    C = -(-counts.max(axis=0) // P)
    # guarantee no (set, tile) is entirely empty (phase-2 reads its psum)
    for i in range(NS):
        for t in range(NT):
            if C[i, t].sum() == 0:
                C[i, t, 0] = 1

    col_of = np.zeros((NS, NT, NB), np.int64)
    blk_col0 = {}   # (s, b) -> first col of the bank block
    blk_cols = {}   # (s, b) -> cols in the block (all sets, span tiles)
    span_col0, span_cols = [], []
    col = 0
    for s in range(cfg.n_spans):
        ts = range(s * GSPAN, min((s + 1) * GSPAN, NT))
        span_col0.append(col)
        for b in range(NB):
            blk_col0[(s, b)] = col
            for i in range(NS):
                for t in ts:
                    col_of[i, t, b] = col
                    col += C[i, t, b]
            blk_cols[(s, b)] = col - blk_col0[(s, b)]
        span_cols.append(col - span_col0[-1])
    totc = col
    tot_slots = totc * P

    idx_mats, dstf_mats = [], []
    for c in range(NC):
        slot_src = np.zeros(tot_slots, np.int64)
        slot_dstf = np.full(tot_slots, -1.0, np.float32)
        for i in range(NS):
            src_s, dloc_s, key_s = per_set[i]
            lo = np.searchsorted(key_s, c * NT * NB)
            hi = np.searchsorted(key_s, (c + 1) * NT * NB)
            src_c, dloc_c, key_c = src_s[lo:hi], dloc_s[lo:hi], key_s[lo:hi]
            t_c = (key_c // NB) % NT
            b_c = key_c % NB
            gstart = np.searchsorted(key_c, key_c)
            rank = np.arange(len(key_c)) - gstart
            slot = col_of[i, t_c, b_c] * P + rank
            slot_src[slot] = src_c - b_c * BANK_ROWS
            slot_dstf[slot] = (dloc_c - t_c * DT).astype(np.float32)
        idx16 = slot_src.reshape(tot_slots // 16, 16).T.astype(np.int16)
        idx_mats.append(np.tile(idx16, (8, 1)))
        dstf_mats.append(slot_dstf.reshape(totc, P).T.copy())

    # gather call list: per (s,b) bank block, split into <=CALL_COLS chunks
    calls = []
    for s in range(cfg.n_spans):
        for b in range(NB):
            cols = blk_cols[(s, b)]
            if cols == 0:
                continue
            g0 = blk_col0[(s, b)]
            nc_split = -(-cols // CALL_COLS)
            step = -(-cols // nc_split)
            for c0 in range(0, cols, step):
                cw = min(step, cols - c0)
                calls.append(dict(s=s, b=b, col0=g0 + c0, cols=cw))
    # queue = ci % 4 (engine parallelism); completion sem = rotating ring of
    # NGSEM per-call sems (exact completion tracking: cumulative multi-queue
    # counts are unsound because the 16 SDMA engines drain independently).
    NGSEM = 16
    slot_rounds = [0] * NGSEM
    for ci, cl in enumerate(calls):
        cl["q"] = ci % 4
        slot = ci % NGSEM
        slot_rounds[slot] += 1
        cl["slot"] = slot
        cl["round"] = slot_rounds[slot]  # 1-based within rep
    group_waits = {}    # (s,b) -> [(slot, round)] for its calls
    span_waits = {}     # s -> {slot: max round} through end of span
    run = {}
    for cl in calls:
        g = (cl["s"], cl["b"])
        group_waits.setdefault(g, []).append((cl["slot"], cl["round"]))
        run[cl["slot"]] = cl["round"]
        span_waits[cl["s"]] = dict(run)

    return dict(C=C, col_of=col_of, totc=totc,
                blk_col0=blk_col0, blk_cols=blk_cols,
                span_col0=span_col0, span_cols=span_cols,
                idx_mats=idx_mats, dstf_mats=dstf_mats,
                calls=calls, group_waits=group_waits,
                span_waits=span_waits, slot_tot=slot_rounds, ngsem=NGSEM)


def _schedule(cfg, prep):
    """Static per-core schedule: ordered chunk list (set-major within span)
    + per-tile completion chunk counts."""
    C, col_of = prep["C"], prep["col_of"]
    NT, NB = cfg.nt, cfg.n_banks
    chunks = []
    tile_chunk_end = {}   # tile t -> chunk count (1-based) at its completion
    chunks_thru_span = {}
    for s in range(cfg.n_spans):
        ts = range(s * GSPAN, min((s + 1) * GSPAN, NT))
        for i in range(NS):
            for t in ts:
                nib = [(b, int(C[i, t, b])) for b in range(NB) if C[i, t, b] > 0]
                tot_k = sum(n for _, n in nib)
                kk = 0
                for b, cc in nib:
                    for k in range(cc):
                        chunks.append(dict(
                            s=s, t=t, i=i, b=b,
                            col=int(col_of[i, t, b]) + k,
                            start=(kk == 0), stop=(kk == tot_k - 1),
                            first_of_group=(k == 0),
                        ))
                        kk += 1
                if i == NS - 1:
                    tile_chunk_end[t] = len(chunks)
        chunks_thru_span[s] = len(chunks)
    return chunks, tile_chunk_end, chunks_thru_span


def _build_kernel(cfg, prep, reps=1, mode="full"):
    # mode: "full" | "gather" (gathers+input DMAs only, for profiling)
    gather_only = (mode == "gather")
    NT, NB = cfg.nt, cfg.n_banks
    calls = prep["calls"]
    group_waits = prep["group_waits"]
    span_waits = prep["span_waits"]
    slot_tot = prep["slot_tot"]
    NGSEM = prep["ngsem"]
    span_col0, span_cols = prep["span_col0"], prep["span_cols"]
    blk_col0, blk_cols = prep["blk_col0"], prep["blk_cols"]
    col_of = prep["col_of"]
    totc = prep["totc"]
    chunks, tile_chunk_end, chunks_thru_span = _schedule(cfg, prep)

    n_chunks = len(chunks)
    n_calls = len(calls)
    n_spans = cfg.n_spans
    max_span_cols = max(span_cols)
    max_blk_cols = max(blk_cols.values())

    pair_idx = {(s, b): s * NB + b for s in range(n_spans) for b in range(NB)}
    out_rows = {(t, h): max(0, min(P, cfg.npc - (t * DT + h * P)))
                for t in range(NT) for h in range(2)}
    n_valid_halves = sum(1 for k in out_rows if out_rows[k] > 0)

    msg_dt = mybir.dt.bfloat16

    nc = bacc.Bacc("TRN2", target_bir_lowering=False, debug=False,
                   num_devices=N_CORES, num_swdge_queues=4)
    x = nc.dram_tensor("x", [cfg.x_rows_pad, P], msg_dt, kind="ExternalInput").ap()
    idx_d = nc.dram_tensor("idx", [P, totc * 8], mybir.dt.int16,
                           kind="ExternalInput").ap()
    dstf_d = nc.dram_tensor("dstf", [P, totc], mybir.dt.float32,
                            kind="ExternalInput").ap()
    iota_d = nc.dram_tensor("iota", [P, DT], msg_dt, kind="ExternalInput").ap()
    w_d = nc.dram_tensor("w", [NS * P, P], mybir.dt.float32,
                         kind="ExternalInput").ap()
    out_d = nc.dram_tensor("out", [cfg.npc, P], mybir.dt.float32,
                           kind="ExternalOutput").ap()

    with (
        nc.Block() as block,
        nc.sbuf_tensor("iota_sb", [P, DT], msg_dt) as iota_sb,
        nc.sbuf_tensor("w_sb", [P, NS, P], mybir.dt.float32) as w_sb,
        nc.sbuf_tensor("idxb", [P, 2, max_span_cols * 8], mybir.dt.int16) as idxb,
        nc.sbuf_tensor("dstfb", [P, 2, max_span_cols], mybir.dt.float32) as dstfb,
        nc.sbuf_tensor("stg", [P, STGRING, max_blk_cols, P], msg_dt) as stg,
        nc.sbuf_tensor("sring", [P, SRING, DT], msg_dt) as sring,
        nc.sbuf_tensor("gsb", [P, 2, NS, DT], mybir.dt.float32) as gsb,
        nc.sbuf_tensor("ot", [P, OTSLOTS, P], mybir.dt.float32) as ot,
        nc.psum_tensor("gp", [P, GPSLOTS, NS, DT], mybir.dt.float32) as gp,
        nc.psum_tensor("op", [P, OTSLOTS, 512], mybir.dt.float32) as op,
        ExitStack() as _stack,
    ):
        _sem = lambda n: _stack.enter_context(nc.semaphore(n))
        cio = _sem("cio")    # const loads done (SP)
        sin = _sem("sin")    # span idx+dstf DMA done (SP, +32/span)
        sgr = [_sem(f"sgr{j}") for j in range(NGSEM)]  # per-call rotating
        sstg = _sem("sstg")  # stg bank blocks consumed by PE (+NB/span)
        ss = _sem("ss")      # one-hot ready (DVE, +1/chunk)
        ssf = _sem("ssf")    # one-hot consumed (PE, +1/chunk)
        st = _sem("st")      # tile psum complete (PE, +1/tile)
        sgp = _sem("sgp")    # gsb ready / gpsum freed (ACT, +1/tile)
        sp2 = _sem("sp2")    # opsum ready (PE, +1/tile)
        sot = _sem("sot")    # ot ready (ACT, +1/tile)
        sof = _sem("sof")    # out DMA done (SP, +16/tile)

        @block.sync
        def _(sy: bass.BassEngine):
            sy.dma_start(iota_sb[:], iota_d[:]).then_inc(cio, 16)
            for i in range(NS):
                sy.dma_start(w_sb[:, i, :],
                             w_d[i * P:(i + 1) * P, :]).then_inc(cio, 16)
            half_seq = 0
            od = 0  # completed-out-DMA self-wait counter

            def emit_out(t):
                nonlocal half_seq, od
                for h in range(2):
                    rows = out_rows[(t, h)]
                    if rows <= 0:
                        half_seq += 1
                        continue
                    sy.wait_ge(sot, half_seq + 1)
                    if od > 0:
                        sy.wait_ge(sof, 16 * od)  # updater-order
                    d0 = t * DT + h * P
                    sy.dma_start(out_d[d0:d0 + rows, :],
                                 ot[:rows, half_seq % OTSLOTS, :]
                                 ).then_inc(sof, 16)
                    od += 1
                    half_seq += 1

            for rep in range(reps):
                for s in range(n_spans):
                    gs = rep * n_spans + s
                    if gs >= 1:
                        sy.wait_ge(sin, 32 * gs)  # updater-order: prior ins done
                    if gs >= 2:
                        ps = gs - 2
                        for j, rnd in sorted(span_waits[ps % n_spans].items()):
                            sy.wait_ge(sgr[j], 16 * ((ps // n_spans)
                                                     * slot_tot[j] + rnd))
                        if not gather_only:
                            sy.wait_ge(ss, (ps // n_spans) * n_chunks
                                       + chunks_thru_span[ps % n_spans])
                    c0, cw = span_col0[s], span_cols[s]
                    sy.dma_start(idxb[:, gs % 2, 0:cw * 8],
                                 idx_d[:, c0 * 8:(c0 + cw) * 8]).then_inc(sin, 16)
                    sy.wait_ge(sin, 32 * gs + 16)
                    sy.dma_start(dstfb[:, gs % 2, 0:cw],
                                 dstf_d[:, c0:c0 + cw]).then_inc(sin, 16)
                    if gs >= 2 and not gather_only:
                        ps_s = (gs - 2) % n_spans
                        for t in range(ps_s * GSPAN,
                                       min((ps_s + 1) * GSPAN, NT)):
                            emit_out(t)
            for gs_tr in range(max(0, reps * n_spans - 2), reps * n_spans):
                if gather_only:
                    break
                s = gs_tr % n_spans
                for t in range(s * GSPAN, min((s + 1) * GSPAN, NT)):
                    emit_out(t)
            for j in range(NGSEM):
                if slot_tot[j] > 0:
                    sy.wait_ge(sgr[j], 16 * reps * slot_tot[j])
            if not gather_only:
                sy.wait_ge(sof, 16 * n_valid_halves * reps)

        @block.gpsimd
        def _(g: bass.BassGpSimd):
            g.load_library(mlp)
            g.wait_ge(cio, 16 * (1 + NS))
            cur_span = -1
            waited_pair = -1
            for rep in range(reps):
                for cl in calls:
                    s, b = cl["s"], cl["b"]
                    gs = rep * n_spans + s
                    gpair = rep * n_spans * NB + pair_idx[(s, b)]
                    if gs != cur_span:
                        g.wait_ge(sin, 32 * (gs + 1))
                        cur_span = gs
                    if (gpair >= STGRING and gpair != waited_pair
                            and not gather_only):
                        # slot reuse: block gpair-STGRING released (releases
                        # are span-batched, NB at a time)
                        g.wait_ge(sstg, NB * ((gpair - STGRING) // NB + 1))
                        waited_pair = gpair
                    k_s = rep * slot_tot[cl["slot"]] + cl["round"]
                    if k_s > 1:
                        g.wait_ge(sgr[cl["slot"]], 16 * (k_s - 1))
                    rel = cl["col0"] - blk_col0[(s, b)]
                    srel = cl["col0"] - span_col0[s]
                    n_idx = cl["cols"] * P
                    g.dma_gather(
                        out_ap=stg[:, gpair % STGRING, rel:rel + cl["cols"], :],
                        in_ap=x[b * BANK_ROWS:(b + 1) * BANK_ROWS, :],
                        idxs_ap=idxb[:, gs % 2, srel * 8:(srel + cl["cols"]) * 8],
                        num_idxs=n_idx,
                        num_idxs_reg=n_idx,
                        elem_size=P,
                        single_packet=False,
                        queue_num=cl["q"],
                    ).then_inc(sgr[cl["slot"]], 16)

        @block.vector
        def _(v: bass.BassVectorEngine):
            if gather_only:
                return
            v.wait_ge(cio, 16 * (1 + NS))
            idx = 0
            cur_span = -1
            for rep in range(reps):
                for ch in chunks:
                    gs = rep * n_spans + ch["s"]
                    if gs != cur_span:
                        v.wait_ge(sin, 32 * (gs + 1))
                        cur_span = gs
                    if idx >= SRING and idx % 4 == 0:
                        v.wait_ge(ssf, idx + 4 - SRING)
                    scol = ch["col"] - span_col0[ch["s"]]
                    v.tensor_scalar(
                        out=sring[:, idx % SRING, :], in0=iota_sb[:],
                        scalar1=dstfb[:, gs % 2, scol:scol + 1],
                        scalar2=None, op0=mybir.AluOpType.is_equal,
                    ).then_inc(ss, 1)
                    idx += 1

        @block.tensor
        def _(t_: bass.BassTensorEngine):
            if gather_only:
                return
            t_.wait_ge(cio, 16 * (1 + NS))
            idx = 0
            tile_seq = 0
            half_seq = 0
            pend = []  # tile_seqs awaiting phase2 (depth 2 for ACT slack)

            def phase2(tseq):
                nonlocal half_seq
                t_.wait_ge(sgp, tseq + 1)
                for h in range(2):
                    if half_seq + 1 > OTSLOTS:
                        t_.wait_ge(sot, half_seq + 1 - OTSLOTS)
                    for i in range(NS):
                        mm = t_.matmul(
                            out=op[:, half_seq % OTSLOTS, 0:P],
                            lhsT=gsb[:, tseq % 2, i, h * P:(h + 1) * P],
                            rhs=w_sb[:, i, :],
                            start=(i == 0), stop=(i == NS - 1))
                        if i == NS - 1:
                            mm.then_inc(sp2, 1)
                    half_seq += 1

            for rep in range(reps):
                last_ti = None
                for ch in chunks:
                    s, t, i, b = ch["s"], ch["t"], ch["i"], ch["b"]
                    # first chunk of tile t overall (i == 0 pass)
                    if i == 0 and (t, rep) != last_ti and ch["start"]:
                        if len(pend) >= 2:
                            phase2(pend.pop(0))
                        ts_new = rep * NT + t
                        if ts_new >= GPSLOTS:
                            t_.wait_ge(sgp, ts_new - GPSLOTS + 1)
                        last_ti = (t, rep)
                    if ch["first_of_group"]:
                        for j, rnd in group_waits[(s, b)]:
                            t_.wait_ge(sgr[j], 16 * (rep * slot_tot[j] + rnd))
                    t_.wait_ge(ss, idx + 1)
                    gpair = rep * n_spans * NB + pair_idx[(s, b)]
                    rel = ch["col"] - blk_col0[(s, b)]
                    ts_cur = rep * NT + t
                    mm = t_.matmul(
                        out=gp[:, ts_cur % GPSLOTS, i, :],
                        lhsT=stg[:, gpair % STGRING, rel, :],
                        rhs=sring[:, idx % SRING, :],
                        start=ch["start"], stop=ch["stop"],
                    )
                    mm.then_inc(ssf, 1)
                    idx += 1
                    if idx - rep * n_chunks == tile_chunk_end[t]:
                        t_.drain().then_inc(st, 1)
                        pend.append(tile_seq)
                        tile_seq += 1
                    if idx - rep * n_chunks == chunks_thru_span[s]:
                        # release all NB bank blocks of span s together
                        for _b in range(NB):
                            t_.drain().then_inc(sstg, 1)
                while pend:
                    phase2(pend.pop(0))

        @block.scalar
        def _(a: bass.BassScalarEngine):
            if gather_only:
                return
            tile_seq = 0
            half_seq = 0
            for rep in range(reps):
                for t in range(NT):
                    a.wait_ge(st, tile_seq + 1)
                    if tile_seq >= 2:
                        # gsb slot reuse: both phase2 halves of tile_seq-2 done
                        a.wait_ge(sp2, 2 * (tile_seq - 1))
                    a.activation(out=gsb[:, tile_seq % 2, :, :],
                                 in_=gp[:, tile_seq % GPSLOTS, :, :],
                                 func=mybir.ActivationFunctionType.Copy
                                 ).then_inc(sgp, 1)
                    for h in range(2):
                        a.wait_ge(sp2, half_seq + 1)
                        if half_seq + 1 > OTSLOTS:
                            a.wait_ge(sof, 16 * (half_seq + 1 - OTSLOTS))
                        a.activation(out=ot[:, half_seq % OTSLOTS, :],
                                     in_=op[:, half_seq % OTSLOTS, 0:P],
                                     func=mybir.ActivationFunctionType.Copy
                                     ).then_inc(sot, 1)
                        half_seq += 1
                    tile_seq += 1

    nc.compile()
    return nc


def _make_in_maps(cfg, prep, x, w_list):
    n_nodes = x.shape[0]
    x_pad = np.zeros((cfg.x_rows_pad, P), np.float32)
    x_pad[:n_nodes] = np.asarray(x, np.float32)
    x_pad = x_pad.astype(ml_dtypes.bfloat16)
    iota = np.tile(np.arange(DT, dtype=np.float32)[None, :], (P, 1)).astype(
        ml_dtypes.bfloat16)
    w_cat = np.concatenate([np.asarray(w, np.float32) for w in w_list], axis=0)

    return [{
        "x": x_pad,
        "idx": prep["idx_mats"][c],
        "dstf": prep["dstf_mats"][c],
        "iota": iota,
        "w": w_cat,
    } for c in range(N_CORES)]


def kernel(hidden_states, edges_i, edges_ii, edges_iii, edges_a,
           W_i, W_ii, W_iii, W_a):
    x = np.asarray(hidden_states, np.float32)
    cfg = _Cfg(x.shape[0])
    edges_list = [np.asarray(e) for e in (edges_i, edges_ii, edges_iii, edges_a)]
    w_list = [W_i, W_ii, W_iii, W_a]

    prep = _host_prep(cfg, edges_list)
    nc = _build_kernel(cfg, prep)
    in_maps = _make_in_maps(cfg, prep, x, w_list)

    res = run_bass_kernel_spmd(nc, in_maps, core_ids=list(range(N_CORES)))
    out = np.concatenate([res.results[c]["out"] for c in range(N_CORES)], axis=0)
    return out.astype(np.float32)


# revision 85
# speedup vs baseline: 1.4204x; 1.1427x over previous
"""GCN message-passing layer (4x GCNConv sum) on 8 Trainium2 NeuronCores.

out[d] = sum_i ( segment_sum_{e in E_i, dst=d} x[src_e] ) @ W_i

Raw-block SPMD kernel (no Tile scheduler): destination nodes sharded across
8 cores, x replicated. Per core:
  - Host groups edges by (core, set, dst-tile(128), src-bank(32768)), sorts
    each group by src (DRAM locality), pads groups to a cross-core-uniform
    multiple of 128 (pad: src=0, dstf=-1).
  - gpsimd dma_gather (int16 bank-local idxs) stages x[src] rows (bf16) into
    SBUF, round-robin over 4 SWDGE queues (4x descriptor throughput).
  - Processing is set-major within each 4-tile span: while PE consumes set
    i's staging, gpsimd prefetches sets i+1, i+2 (stg ring of 3).
  - DVE builds one-hot S[e,d] = (dstf[e]==d) per 128-edge chunk; TensorE
    accumulates g_i^T = stg^T @ S into per-tile PSUM (1 bank, 6-slot ring).
  - ACT copies tile PSUM->SBUF; TensorE phase 2: out_tile = sum_i gsb_i^T.T
    @ W_i into opsum; ACT copies opsum->SBUF; SP DMAs out rows.
All cross-engine sync is explicit semaphores; every core runs an identical
program (counts are cross-core uniform by construction).
"""
import math
import sys
from contextlib import ExitStack

sys.path.insert(0, "/opt/trn_rl_repo")

import numpy as np
import ml_dtypes

from concourse import bass, mybir, bacc
from concourse.bass_utils import run_bass_kernel_spmd
from concourse.library_config import mlp

P = 128
N_CORES = 8
DT = 256           # dst-tile width (two PSUM banks per (tile, 4 sets))
BANK_ROWS = 32768
NS = 4
GSPAN = 2          # tiles per idx/gather span (must be <= GPSLOTS)
CALL_COLS = 96     # max 128-edge chunks per dma_gather call (12288 idxs)
SRING = 8          # one-hot ring slots
STGRING = 7        # stg (span,bank) block ring slots (> NB)
GPSLOTS = 3        # gpsum PSUM rotation (2 banks each)
OTSLOTS = 2        # opsum/ot half-tile rotation (1 PSUM bank per slot)


class _Cfg:
    def __init__(self, n_nodes):
        self.n_nodes = n_nodes
        self.npc = n_nodes // N_CORES
        self.nt = math.ceil(self.npc / DT)
        self.n_banks = math.ceil(n_nodes / BANK_ROWS)
        self.x_rows_pad = self.n_banks * BANK_ROWS
        self.n_spans = math.ceil(self.nt / GSPAN)


def _host_prep(cfg, edges_list):
    NC, NT, NB = N_CORES, cfg.nt, cfg.n_banks
    counts = np.zeros((NC, NS, NT, NB), np.int64)
    per_set = []
    for i, e in enumerate(edges_list):
        src = np.asarray(e[0], np.int64)
        dst = np.asarray(e[1], np.int64)
        core = dst // cfg.npc
        dloc = dst % cfg.npc
        t = dloc // DT
        b = src // BANK_ROWS
        key = (core * NT + t) * NB + b
        counts[:, i] = np.bincount(key, minlength=NC * NT * NB).reshape(NC, NT, NB)
        order = np.argsort(key, kind="stable")
        per_set.append((src[order], dloc[order], key[order]))

    C = -(-counts.max(axis=0) // P)
    # guarantee no (set, tile) is entirely empty (phase-2 reads its psum)
    for i in range(NS):
        for t in range(NT):
            if C[i, t].sum() == 0:
                C[i, t, 0] = 1

    col_of = np.zeros((NS, NT, NB), np.int64)
    blk_col0 = {}   # (s, b) -> first col of the bank block
    blk_cols = {}   # (s, b) -> cols in the block (all sets, span tiles)
    span_col0, span_cols = [], []
    col = 0
    for s in range(cfg.n_spans):
        ts = range(s * GSPAN, min((s + 1) * GSPAN, NT))
        span_col0.append(col)
        for b in range(NB):
            blk_col0[(s, b)] = col
            for i in range(NS):
                for t in ts:
                    col_of[i, t, b] = col
                    col += C[i, t, b]
            blk_cols[(s, b)] = col - blk_col0[(s, b)]
        span_cols.append(col - span_col0[-1])
    totc = col
    tot_slots = totc * P

    idx_mats, dstf_mats = [], []
    for c in range(NC):
        slot_src = np.zeros(tot_slots, np.int64)
        slot_dstf = np.full(tot_slots, -1.0, np.float32)
        for i in range(NS):
            src_s, dloc_s, key_s = per_set[i]
            lo = np.searchsorted(key_s, c * NT * NB)
            hi = np.searchsorted(key_s, (c + 1) * NT * NB)
            src_c, dloc_c, key_c = src_s[lo:hi], dloc_s[lo:hi], key_s[lo:hi]
            t_c = (key_c // NB) % NT
            b_c = key_c % NB
            gstart = np.searchsorted(key_c, key_c)
            rank = np.arange(len(key_c)) - gstart
            slot = col_of[i, t_c, b_c] * P + rank
            slot_src[slot] = src_c - b_c * BANK_ROWS
            slot_dstf[slot] = (dloc_c - t_c * DT).astype(np.float32)
        idx16 = slot_src.reshape(tot_slots // 16, 16).T.astype(np.int16)
        idx_mats.append(np.tile(idx16, (8, 1)))
        dstf_mats.append(slot_dstf.reshape(totc, P).T.copy())

    # gather call list: per (s,b) bank block, split into <=CALL_COLS chunks
    calls = []
    for s in range(cfg.n_spans):
        for b in range(NB):
            cols = blk_cols[(s, b)]
            if cols == 0:
                continue
            g0 = blk_col0[(s, b)]
            nc_split = -(-cols // CALL_COLS)
            step = -(-cols // nc_split)
            for c0 in range(0, cols, step):
                cw = min(step, cols - c0)
                calls.append(dict(s=s, b=b, col0=g0 + c0, cols=cw))
    # queue = ci % 4 (engine parallelism); completion sem = rotating ring of
    # NGSEM per-call sems (exact completion tracking: cumulative multi-queue
    # counts are unsound because the 16 SDMA engines drain independently).
    NGSEM = 16
    slot_rounds = [0] * NGSEM
    for ci, cl in enumerate(calls):
        cl["q"] = ci % 4
        slot = ci % NGSEM
        slot_rounds[slot] += 1
        cl["slot"] = slot
        cl["round"] = slot_rounds[slot]  # 1-based within rep
    group_waits = {}    # (s,b) -> [(slot, round)] for its calls
    span_waits = {}     # s -> {slot: max round} through end of span
    run = {}
    for cl in calls:
        g = (cl["s"], cl["b"])
        group_waits.setdefault(g, []).append((cl["slot"], cl["round"]))
        run[cl["slot"]] = cl["round"]
        span_waits[cl["s"]] = dict(run)

    return dict(C=C, col_of=col_of, totc=totc,
                blk_col0=blk_col0, blk_cols=blk_cols,
                span_col0=span_col0, span_cols=span_cols,
                idx_mats=idx_mats, dstf_mats=dstf_mats,
                calls=calls, group_waits=group_waits,
                span_waits=span_waits, slot_tot=slot_rounds, ngsem=NGSEM)


def _schedule(cfg, prep):
    """Static per-core schedule: ordered chunk list (set-major within span)
    + per-tile completion chunk counts."""
    C, col_of = prep["C"], prep["col_of"]
    NT, NB = cfg.nt, cfg.n_banks
    chunks = []
    tile_chunk_end = {}   # tile t -> chunk count (1-based) at its completion
    chunks_thru_span = {}
    for s in range(cfg.n_spans):
        ts = range(s * GSPAN, min((s + 1) * GSPAN, NT))
        for i in range(NS):
            for t in ts:
                nib = [(b, int(C[i, t, b])) for b in range(NB) if C[i, t, b] > 0]
                tot_k = sum(n for _, n in nib)
                kk = 0
                for b, cc in nib:
                    for k in range(cc):
                        chunks.append(dict(
                            s=s, t=t, i=i, b=b,
                            col=int(col_of[i, t, b]) + k,
                            start=(kk == 0), stop=(kk == tot_k - 1),
                            first_of_group=(k == 0),
                        ))
                        kk += 1
                if i == NS - 1:
                    tile_chunk_end[t] = len(chunks)
        chunks_thru_span[s] = len(chunks)
    return chunks, tile_chunk_end, chunks_thru_span


def _build_kernel(cfg, prep, reps=1, mode="full"):
    # mode: "full" | "gather" (gathers+input DMAs only, for profiling)
    gather_only = (mode == "gather")
    NT, NB = cfg.nt, cfg.n_banks
    calls = prep["calls"]
    group_waits = prep["group_waits"]
    span_waits = prep["span_waits"]
    slot_tot = prep["slot_tot"]
    NGSEM = prep["ngsem"]
    span_col0, span_cols = prep["span_col0"], prep["span_cols"]
    blk_col0, blk_cols = prep["blk_col0"], prep["blk_cols"]
    col_of = prep["col_of"]
    totc = prep["totc"]
    chunks, tile_chunk_end, chunks_thru_span = _schedule(cfg, prep)

    n_chunks = len(chunks)
    n_calls = len(calls)
    n_spans = cfg.n_spans
    max_span_cols = max(span_cols)
    max_blk_cols = max(blk_cols.values())

    pair_idx = {(s, b): s * NB + b for s in range(n_spans) for b in range(NB)}
    out_rows = {(t, h): max(0, min(P, cfg.npc - (t * DT + h * P)))
                for t in range(NT) for h in range(2)}
    n_valid_halves = sum(1 for k in out_rows if out_rows[k] > 0)

    msg_dt = mybir.dt.bfloat16

    nc = bacc.Bacc("TRN2", target_bir_lowering=False, debug=False,
                   num_devices=N_CORES, num_swdge_queues=4)
    x = nc.dram_tensor("x", [cfg.x_rows_pad, P], msg_dt, kind="ExternalInput").ap()
    idx_d = nc.dram_tensor("idx", [P, totc * 8], mybir.dt.int16,
                           kind="ExternalInput").ap()
    dstf_d = nc.dram_tensor("dstf", [P, totc], mybir.dt.float32,
                            kind="ExternalInput").ap()
    iota_d = nc.dram_tensor("iota", [P, DT], msg_dt, kind="ExternalInput").ap()
    w_d = nc.dram_tensor("w", [NS * P, P], mybir.dt.float32,
                         kind="ExternalInput").ap()
    out_d = nc.dram_tensor("out", [cfg.npc, P], mybir.dt.float32,
                           kind="ExternalOutput").ap()

    with (
        nc.Block() as block,
        nc.sbuf_tensor("iota_sb", [P, DT], msg_dt) as iota_sb,
        nc.sbuf_tensor("w_sb", [P, NS, P], mybir.dt.float32) as w_sb,
        nc.sbuf_tensor("idxb", [P, 2, max_span_cols * 8], mybir.dt.int16) as idxb,
        nc.sbuf_tensor("dstfb", [P, 2, max_span_cols], mybir.dt.float32) as dstfb,
        nc.sbuf_tensor("stg", [P, STGRING, max_blk_cols, P], msg_dt) as stg,
        nc.sbuf_tensor("sring", [P, SRING, DT], msg_dt) as sring,
        nc.sbuf_tensor("gsb", [P, 2, NS, DT], mybir.dt.float32) as gsb,
        nc.sbuf_tensor("ot", [P, OTSLOTS, P], mybir.dt.float32) as ot,
        nc.psum_tensor("gp", [P, GPSLOTS, NS, DT], mybir.dt.float32) as gp,
        nc.psum_tensor("op", [P, OTSLOTS, 512], mybir.dt.float32) as op,
        ExitStack() as _stack,
    ):
        _sem = lambda n: _stack.enter_context(nc.semaphore(n))
        cio = _sem("cio")    # const loads done (SP)
        sin = _sem("sin")    # span idx+dstf DMA done (SP, +32/span)
        sgr = [_sem(f"sgr{j}") for j in range(NGSEM)]  # per-call rotating
        sstg = _sem("sstg")  # stg bank blocks consumed by PE (+NB/span)
        ss = _sem("ss")      # one-hot ready (DVE, +1/chunk)
        ssf = _sem("ssf")    # one-hot consumed (PE, +1/chunk)
        st = _sem("st")      # tile psum complete (PE, +1/tile)
        sgp = _sem("sgp")    # gsb ready / gpsum freed (ACT, +1/tile)
        sp2 = _sem("sp2")    # opsum ready (PE, +1/tile)
        sot = _sem("sot")    # ot ready (ACT, +1/tile)
        sof = _sem("sof")    # out DMA done (SP, +16/tile)

        @block.sync
        def _(sy: bass.BassEngine):
            sy.dma_start(iota_sb[:], iota_d[:]).then_inc(cio, 16)
            for i in range(NS):
                sy.dma_start(w_sb[:, i, :],
                             w_d[i * P:(i + 1) * P, :]).then_inc(cio, 16)
            half_seq = 0
            od = 0  # completed-out-DMA self-wait counter

            def emit_out(t):
                nonlocal half_seq, od
                for h in range(2):
                    rows = out_rows[(t, h)]
                    if rows <= 0:
                        half_seq += 1
                        continue
                    sy.wait_ge(sot, half_seq + 1)
                    if od > 0:
                        sy.wait_ge(sof, 16 * od)  # updater-order
                    d0 = t * DT + h * P
                    sy.dma_start(out_d[d0:d0 + rows, :],
                                 ot[:rows, half_seq % OTSLOTS, :]
                                 ).then_inc(sof, 16)
                    od += 1
                    half_seq += 1

            for rep in range(reps):
                for s in range(n_spans):
                    gs = rep * n_spans + s
                    if gs >= 1:
                        sy.wait_ge(sin, 32 * gs)  # updater-order: prior ins done
                    if gs >= 2:
                        ps = gs - 2
                        for j, rnd in sorted(span_waits[ps % n_spans].items()):
                            sy.wait_ge(sgr[j], 16 * ((ps // n_spans)
                                                     * slot_tot[j] + rnd))
                        if not gather_only:
                            sy.wait_ge(ss, (ps // n_spans) * n_chunks
                                       + chunks_thru_span[ps % n_spans])
                    c0, cw = span_col0[s], span_cols[s]
                    sy.dma_start(idxb[:, gs % 2, 0:cw * 8],
                                 idx_d[:, c0 * 8:(c0 + cw) * 8]).then_inc(sin, 16)
                    sy.wait_ge(sin, 32 * gs + 16)
                    sy.dma_start(dstfb[:, gs % 2, 0:cw],
                                 dstf_d[:, c0:c0 + cw]).then_inc(sin, 16)
                    if gs >= 2 and not gather_only:
                        ps_s = (gs - 2) % n_spans
                        for t in range(ps_s * GSPAN,
                                       min((ps_s + 1) * GSPAN, NT)):
                            emit_out(t)
            for gs_tr in range(max(0, reps * n_spans - 2), reps * n_spans):
                if gather_only:
                    break
                s = gs_tr % n_spans
                for t in range(s * GSPAN, min((s + 1) * GSPAN, NT)):
                    emit_out(t)
            for j in range(NGSEM):
                if slot_tot[j] > 0:
                    sy.wait_ge(sgr[j], 16 * reps * slot_tot[j])
            if not gather_only:
                sy.wait_ge(sof, 16 * n_valid_halves * reps)

        @block.gpsimd
        def _(g: bass.BassGpSimd):
            g.load_library(mlp)
            g.wait_ge(cio, 16 * (1 + NS))
            cur_span = -1
            waited_pair = -1
            for rep in range(reps):
                for cl in calls:
                    s, b = cl["s"], cl["b"]
                    gs = rep * n_spans + s
                    gpair = rep * n_spans * NB + pair_idx[(s, b)]
                    if gs != cur_span:
                        g.wait_ge(sin, 32 * (gs + 1))
                        cur_span = gs
                    if (gpair >= STGRING and gpair != waited_pair
                            and not gather_only):
                        # slot reuse: block gpair-STGRING released (releases
                        # are span-batched, NB at a time)
                        g.wait_ge(sstg, NB * ((gpair - STGRING) // NB + 1))
                        waited_pair = gpair
                    k_s = rep * slot_tot[cl["slot"]] + cl["round"]
                    if k_s > 1:
                        g.wait_ge(sgr[cl["slot"]], 16 * (k_s - 1))
                    rel = cl["col0"] - blk_col0[(s, b)]
                    srel = cl["col0"] - span_col0[s]
                    n_idx = cl["cols"] * P
                    g.dma_gather(
                        out_ap=stg[:, gpair % STGRING, rel:rel + cl["cols"], :],
                        in_ap=x[b * BANK_ROWS:(b + 1) * BANK_ROWS, :],
                        idxs_ap=idxb[:, gs % 2, srel * 8:(srel + cl["cols"]) * 8],
                        num_idxs=n_idx,
                        num_idxs_reg=n_idx,
                        elem_size=P,
                        single_packet=False,
                        queue_num=cl["q"],
                    ).then_inc(sgr[cl["slot"]], 16)

        @block.vector
        def _(v: bass.BassVectorEngine):
            if gather_only:
                return
            v.wait_ge(cio, 16 * (1 + NS))
            idx = 0
            cur_span = -1
            for rep in range(reps):
                for ch in chunks:
                    gs = rep * n_spans + ch["s"]
                    if gs != cur_span:
                        v.wait_ge(sin, 32 * (gs + 1))
                        cur_span = gs
                    if idx >= SRING and idx % 4 == 0:
                        v.wait_ge(ssf, idx + 4 - SRING)
                    scol = ch["col"] - span_col0[ch["s"]]
                    v.tensor_scalar(
                        out=sring[:, idx % SRING, :], in0=iota_sb[:],
                        scalar1=dstfb[:, gs % 2, scol:scol + 1],
                        scalar2=None, op0=mybir.AluOpType.is_equal,
                    ).then_inc(ss, 1)
                    idx += 1

        @block.tensor
        def _(t_: bass.BassTensorEngine):
            if gather_only:
                return
            t_.wait_ge(cio, 16 * (1 + NS))
            idx = 0
            tile_seq = 0
            half_seq = 0
            pend = []  # tile_seqs awaiting phase2 (depth 2 for ACT slack)

            def phase2(tseq):
                nonlocal half_seq
                t_.wait_ge(sgp, tseq + 1)
                for h in range(2):
                    if half_seq + 1 > OTSLOTS:
                        t_.wait_ge(sot, half_seq + 1 - OTSLOTS)
                    for i in range(NS):
                        mm = t_.matmul(
                            out=op[:, half_seq % OTSLOTS, 0:P],
                            lhsT=gsb[:, tseq % 2, i, h * P:(h + 1) * P],
                            rhs=w_sb[:, i, :],
                            start=(i == 0), stop=(i == NS - 1))
                        if i == NS - 1:
                            mm.then_inc(sp2, 1)
                    half_seq += 1

            for rep in range(reps):
                last_ti = None
                for ch in chunks:
                    s, t, i, b = ch["s"], ch["t"], ch["i"], ch["b"]
                    # first chunk of tile t overall (i == 0 pass)
                    if i == 0 and (t, rep) != last_ti and ch["start"]:
                        if len(pend) >= 2:
                            phase2(pend.pop(0))
                        ts_new = rep * NT + t
                        if ts_new >= GPSLOTS:
                            t_.wait_ge(sgp, ts_new - GPSLOTS + 1)
                        last_ti = (t, rep)
                    if ch["first_of_group"]:
                        for j, rnd in group_waits[(s, b)]:
                            t_.wait_ge(sgr[j], 16 * (rep * slot_tot[j] + rnd))
                    t_.wait_ge(ss, idx + 1)
                    gpair = rep * n_spans * NB + pair_idx[(s, b)]
                    rel = ch["col"] - blk_col0[(s, b)]
                    ts_cur = rep * NT + t
                    mm = t_.matmul(
                        out=gp[:, ts_cur % GPSLOTS, i, :],
                        lhsT=stg[:, gpair % STGRING, rel, :],
                        rhs=sring[:, idx % SRING, :],
                        start=ch["start"], stop=ch["stop"],
                    )
                    mm.then_inc(ssf, 1)
                    idx += 1
                    if idx - rep * n_chunks == tile_chunk_end[t]:
                        t_.drain().then_inc(st, 1)
                        pend.append(tile_seq)
                        tile_seq += 1
                    if idx - rep * n_chunks == chunks_thru_span[s]:
                        # release all NB bank blocks of span s together
                        for _b in range(NB):
                            t_.drain().then_inc(sstg, 1)
                while pend:
                    phase2(pend.pop(0))

        @block.scalar
        def _(a: bass.BassScalarEngine):
            if gather_only:
                return
            tile_seq = 0
            half_seq = 0
            for rep in range(reps):
                for t in range(NT):
                    a.wait_ge(st, tile_seq + 1)
                    if tile_seq >= 2:
                        # gsb slot reuse: both phase2 halves of tile_seq-2 done
                        a.wait_ge(sp2, 2 * (tile_seq - 1))
                    a.activation(out=gsb[:, tile_seq % 2, :, :],
                                 in_=gp[:, tile_seq % GPSLOTS, :, :],
                                 func=mybir.ActivationFunctionType.Copy
                                 ).then_inc(sgp, 1)
                    for h in range(2):
                        a.wait_ge(sp2, half_seq + 1)
                        if half_seq + 1 > OTSLOTS:
                            a.wait_ge(sof, 16 * (half_seq + 1 - OTSLOTS))
                        a.activation(out=ot[:, half_seq % OTSLOTS, :],
                                     in_=op[:, half_seq % OTSLOTS, 0:P],
                                     func=mybir.ActivationFunctionType.Copy
                                     ).then_inc(sot, 1)
                        half_seq += 1
                    tile_seq += 1

    nc.compile()
    return nc


def _make_in_maps(cfg, prep, x, w_list):
    n_nodes = x.shape[0]
    x_pad = np.zeros((cfg.x_rows_pad, P), np.float32)
    x_pad[:n_nodes] = np.asarray(x, np.float32)
    x_pad = x_pad.astype(ml_dtypes.bfloat16)
    iota = np.tile(np.arange(DT, dtype=np.float32)[None, :], (P, 1)).astype(
        ml_dtypes.bfloat16)
    w_cat = np.concatenate([np.asarray(w, np.float32) for w in w_list], axis=0)

    return [{
        "x": x_pad,
        "idx": prep["idx_mats"][c],
        "dstf": prep["dstf_mats"][c],
        "iota": iota,
        "w": w_cat,
    } for c in range(N_CORES)]


def kernel(hidden_states, edges_i, edges_ii, edges_iii, edges_a,
           W_i, W_ii, W_iii, W_a):
    x = np.asarray(hidden_states, np.float32)
    cfg = _Cfg(x.shape[0])
    edges_list = [np.asarray(e) for e in (edges_i, edges_ii, edges_iii, edges_a)]
    w_list = [W_i, W_ii, W_iii, W_a]

    prep = _host_prep(cfg, edges_list)
    nc = _build_kernel(cfg, prep)
    in_maps = _make_in_maps(cfg, prep, x, w_list)

    res = run_bass_kernel_spmd(nc, in_maps, core_ids=list(range(N_CORES)))
    out = np.concatenate([res.results[c]["out"] for c in range(N_CORES)], axis=0)
    return out.astype(np.float32)


# revision 100
# speedup vs baseline: 1.6008x; 1.1270x over previous
"""GCN message-passing layer (4x GCNConv sum) on 8 Trainium2 NeuronCores.

out[d] = sum_i ( segment_sum_{e in E_i, dst=d} x[src_e] ) @ W_i

Raw-block SPMD kernel (no Tile scheduler): destination nodes sharded across
8 cores, x replicated. Per core:
  - Host groups edges by (core, set, dst-tile(128), src-bank(32768)), sorts
    each group by src (DRAM locality), pads groups to a cross-core-uniform
    multiple of 128 (pad: src=0, dstf=-1).
  - gpsimd dma_gather (int16 bank-local idxs) stages x[src] rows (bf16) into
    SBUF, round-robin over 4 SWDGE queues (4x descriptor throughput).
  - Processing is set-major within each 4-tile span: while PE consumes set
    i's staging, gpsimd prefetches sets i+1, i+2 (stg ring of 3).
  - DVE builds one-hot S[e,d] = (dstf[e]==d) per 128-edge chunk; TensorE
    accumulates g_i^T = stg^T @ S into per-tile PSUM (1 bank, 6-slot ring).
  - ACT copies tile PSUM->SBUF; TensorE phase 2: out_tile = sum_i gsb_i^T.T
    @ W_i into opsum; ACT copies opsum->SBUF; SP DMAs out rows.
All cross-engine sync is explicit semaphores; every core runs an identical
program (counts are cross-core uniform by construction).
"""
import math
import sys
from contextlib import ExitStack

sys.path.insert(0, "/opt/trn_rl_repo")

import numpy as np
import ml_dtypes

from concourse import bass, mybir, bacc
from concourse.bass_utils import run_bass_kernel_spmd
from concourse.library_config import mlp

P = 128
N_CORES = 8
DT = 256           # dst-tile width (two PSUM banks per (tile, 4 sets))
BANK_ROWS = 32768
NS = 4
GSPAN = 2          # tiles per idx/gather span (must be <= GPSLOTS)
CALL_COLS = 96     # max 128-edge chunks per dma_gather call (12288 idxs)
SRING = 8          # one-hot ring slots (multiple of 4: batched ssf waits)
STGRING = 8        # stg (span,bank) block ring slots (> NB)
GPSLOTS = 3        # gpsum PSUM rotation (2 banks each)
OTSLOTS = 2        # opsum/ot half-tile rotation (1 PSUM bank per slot)


class _Cfg:
    def __init__(self, n_nodes):
        self.n_nodes = n_nodes
        self.npc = n_nodes // N_CORES
        self.nt = math.ceil(self.npc / DT)
        self.n_banks = math.ceil(n_nodes / BANK_ROWS)
        self.x_rows_pad = self.n_banks * BANK_ROWS
        self.n_spans = math.ceil(self.nt / GSPAN)


def _host_prep(cfg, edges_list):
    NC, NT, NB = N_CORES, cfg.nt, cfg.n_banks
    counts = np.zeros((NC, NS, NT, NB), np.int64)
    per_set = []
    for i, e in enumerate(edges_list):
        src = np.asarray(e[0], np.int64)
        dst = np.asarray(e[1], np.int64)
        core = dst // cfg.npc
        dloc = dst % cfg.npc
        t = dloc // DT
        b = src // BANK_ROWS
        key = (core * NT + t) * NB + b
        counts[:, i] = np.bincount(key, minlength=NC * NT * NB).reshape(NC, NT, NB)
        order = np.argsort(key, kind="stable")
        per_set.append((src[order], dloc[order], key[order]))

    C = -(-counts.max(axis=0) // P)
    # guarantee no (set, tile) is entirely empty (phase-2 reads its psum)
    for i in range(NS):
        for t in range(NT):
            if C[i, t].sum() == 0:
                C[i, t, 0] = 1

    col_of = np.zeros((NS, NT, NB), np.int64)
    blk_col0 = {}   # (s, b) -> first col of the bank block
    blk_cols = {}   # (s, b) -> cols in the block (all sets, span tiles)
    span_col0, span_cols = [], []
    col = 0
    for s in range(cfg.n_spans):
        ts = range(s * GSPAN, min((s + 1) * GSPAN, NT))
        span_col0.append(col)
        for b in range(NB):
            blk_col0[(s, b)] = col
            for i in range(NS):
                for t in ts:
                    col_of[i, t, b] = col
                    col += C[i, t, b]
            blk_cols[(s, b)] = col - blk_col0[(s, b)]
        span_cols.append(col - span_col0[-1])
    totc = col
    tot_slots = totc * P

    idx_mats, dstf_mats = [], []
    for c in range(NC):
        slot_src = np.zeros(tot_slots, np.int64)
        slot_dstf = np.full(tot_slots, -1.0, np.float32)
        for i in range(NS):
            src_s, dloc_s, key_s = per_set[i]
            lo = np.searchsorted(key_s, c * NT * NB)
            hi = np.searchsorted(key_s, (c + 1) * NT * NB)
            src_c, dloc_c, key_c = src_s[lo:hi], dloc_s[lo:hi], key_s[lo:hi]
            t_c = (key_c // NB) % NT
            b_c = key_c % NB
            gstart = np.searchsorted(key_c, key_c)
            rank = np.arange(len(key_c)) - gstart
            slot = col_of[i, t_c, b_c] * P + rank
            slot_src[slot] = src_c - b_c * BANK_ROWS
            slot_dstf[slot] = (dloc_c - t_c * DT).astype(np.float32)
        idx16 = slot_src.reshape(tot_slots // 16, 16).T.astype(np.int16)
        idx_mats.append(np.tile(idx16, (8, 1)))
        dstf_mats.append(slot_dstf.reshape(totc, P).T.copy())

    # gather call list: per (s,b) bank block, split into <=CALL_COLS chunks
    calls = []
    for s in range(cfg.n_spans):
        for b in range(NB):
            cols = blk_cols[(s, b)]
            if cols == 0:
                continue
            g0 = blk_col0[(s, b)]
            nc_split = -(-cols // CALL_COLS)
            step = -(-cols // nc_split)
            for c0 in range(0, cols, step):
                cw = min(step, cols - c0)
                calls.append(dict(s=s, b=b, col0=g0 + c0, cols=cw))
    # queue = ci % 4 (engine parallelism); completion sem = rotating ring of
    # NGSEM per-call sems (exact completion tracking: cumulative multi-queue
    # counts are unsound because the 16 SDMA engines drain independently).
    NGSEM = 16
    slot_rounds = [0] * NGSEM
    for ci, cl in enumerate(calls):
        cl["q"] = ci % 4
        slot = ci % NGSEM
        slot_rounds[slot] += 1
        cl["slot"] = slot
        cl["round"] = slot_rounds[slot]  # 1-based within rep
    group_waits = {}    # (s,b) -> [(slot, round)] for its calls
    span_waits = {}     # s -> {slot: max round} through end of span
    run = {}
    for cl in calls:
        g = (cl["s"], cl["b"])
        group_waits.setdefault(g, []).append((cl["slot"], cl["round"]))
        run[cl["slot"]] = cl["round"]
        span_waits[cl["s"]] = dict(run)

    return dict(C=C, col_of=col_of, totc=totc,
                blk_col0=blk_col0, blk_cols=blk_cols,
                span_col0=span_col0, span_cols=span_cols,
                idx_mats=idx_mats, dstf_mats=dstf_mats,
                calls=calls, group_waits=group_waits,
                span_waits=span_waits, slot_tot=slot_rounds, ngsem=NGSEM)


def _schedule(cfg, prep):
    """Static per-core schedule: ordered chunk list (set-major within span)
    + per-tile completion chunk counts."""
    C, col_of = prep["C"], prep["col_of"]
    NT, NB = cfg.nt, cfg.n_banks
    chunks = []
    tile_chunk_end = {}   # tile t -> chunk count (1-based) at its completion
    chunks_thru_span = {}
    for s in range(cfg.n_spans):
        ts = range(s * GSPAN, min((s + 1) * GSPAN, NT))
        for i in range(NS):
            for t in ts:
                nib = [(b, int(C[i, t, b])) for b in range(NB) if C[i, t, b] > 0]
                tot_k = sum(n for _, n in nib)
                kk = 0
                for b, cc in nib:
                    for k in range(cc):
                        chunks.append(dict(
                            s=s, t=t, i=i, b=b,
                            col=int(col_of[i, t, b]) + k,
                            start=(kk == 0), stop=(kk == tot_k - 1),
                            first_of_group=(k == 0),
                        ))
                        kk += 1
                if i == NS - 1:
                    tile_chunk_end[t] = len(chunks)
        chunks_thru_span[s] = len(chunks)
    return chunks, tile_chunk_end, chunks_thru_span


def _build_kernel(cfg, prep, reps=1, mode="full"):
    # mode: "full" | "gather" (gathers+input DMAs only, for profiling)
    gather_only = (mode == "gather")
    NT, NB = cfg.nt, cfg.n_banks
    calls = prep["calls"]
    group_waits = prep["group_waits"]
    span_waits = prep["span_waits"]
    slot_tot = prep["slot_tot"]
    NGSEM = prep["ngsem"]
    span_col0, span_cols = prep["span_col0"], prep["span_cols"]
    blk_col0, blk_cols = prep["blk_col0"], prep["blk_cols"]
    col_of = prep["col_of"]
    totc = prep["totc"]
    chunks, tile_chunk_end, chunks_thru_span = _schedule(cfg, prep)

    n_chunks = len(chunks)
    n_calls = len(calls)
    n_spans = cfg.n_spans
    max_span_cols = max(span_cols)
    max_blk_cols = max(blk_cols.values())

    pair_idx = {(s, b): s * NB + b for s in range(n_spans) for b in range(NB)}
    out_rows = {(t, h): max(0, min(P, cfg.npc - (t * DT + h * P)))
                for t in range(NT) for h in range(2)}
    n_valid_halves = sum(1 for k in out_rows if out_rows[k] > 0)

    msg_dt = mybir.dt.bfloat16

    nc = bacc.Bacc("TRN2", target_bir_lowering=False, debug=False,
                   num_devices=N_CORES, num_swdge_queues=4)
    x = nc.dram_tensor("x", [cfg.x_rows_pad, P], msg_dt, kind="ExternalInput").ap()
    idx_d = nc.dram_tensor("idx", [P, totc * 8], mybir.dt.int16,
                           kind="ExternalInput").ap()
    dstf_d = nc.dram_tensor("dstf", [P, totc], mybir.dt.float32,
                            kind="ExternalInput").ap()
    iota_d = nc.dram_tensor("iota", [P, DT], msg_dt, kind="ExternalInput").ap()
    w_d = nc.dram_tensor("w", [NS * P, P], msg_dt,
                         kind="ExternalInput").ap()
    out_d = nc.dram_tensor("out", [cfg.npc, P], msg_dt,
                           kind="ExternalOutput").ap()

    with (
        nc.Block() as block,
        nc.sbuf_tensor("iota_sb", [P, DT], msg_dt) as iota_sb,
        nc.sbuf_tensor("w_sb", [P, NS, P], msg_dt) as w_sb,
        nc.sbuf_tensor("idxb", [P, 2, max_span_cols * 8], mybir.dt.int16) as idxb,
        nc.sbuf_tensor("dstfb", [P, 2, max_span_cols], mybir.dt.float32) as dstfb,
        nc.sbuf_tensor("stg", [P, STGRING, max_blk_cols, P], msg_dt) as stg,
        nc.sbuf_tensor("sring", [P, SRING, DT], msg_dt) as sring,
        nc.sbuf_tensor("gsb", [P, 2, NS, DT], msg_dt) as gsb,
        nc.sbuf_tensor("ot", [P, OTSLOTS, P], msg_dt) as ot,
        nc.psum_tensor("gp", [P, GPSLOTS, NS, DT], mybir.dt.float32) as gp,
        nc.psum_tensor("op", [P, OTSLOTS, 512], mybir.dt.float32) as op,
        ExitStack() as _stack,
    ):
        _sem = lambda n: _stack.enter_context(nc.semaphore(n))
        cio = _sem("cio")    # const loads done (SP)
        sin = _sem("sin")    # span idx+dstf DMA done (SP, +32/span)
        sgr = [_sem(f"sgr{j}") for j in range(NGSEM)]  # per-call rotating
        sstg = _sem("sstg")  # stg bank blocks consumed by PE (+NB/span)
        ss = _sem("ss")      # one-hot ready (DVE, +1/chunk)
        ssf = _sem("ssf")    # one-hot consumed (PE, +1/chunk)
        st = _sem("st")      # tile psum complete (PE, +1/tile)
        sgp = _sem("sgp")    # gsb ready / gpsum freed (ACT, +1/tile)
        sp2 = _sem("sp2")    # opsum ready (PE, +1/tile)
        sot = _sem("sot")    # ot ready (ACT, +1/tile)
        sof = _sem("sof")    # out DMA done (SP, +16/tile)

        @block.sync
        def _(sy: bass.BassEngine):
            sy.dma_start(iota_sb[:], iota_d[:]).then_inc(cio, 16)
            for i in range(NS):
                sy.dma_start(w_sb[:, i, :],
                             w_d[i * P:(i + 1) * P, :]).then_inc(cio, 16)
            half_seq = 0
            od = 0  # completed-out-DMA self-wait counter

            def emit_out(t):
                nonlocal half_seq, od
                for h in range(2):
                    rows = out_rows[(t, h)]
                    if rows <= 0:
                        half_seq += 1
                        continue
                    sy.wait_ge(sot, half_seq + 1)
                    if od > 0:
                        sy.wait_ge(sof, 16 * od)  # updater-order
                    d0 = t * DT + h * P
                    sy.dma_start(out_d[d0:d0 + rows, :],
                                 ot[:rows, half_seq % OTSLOTS, :]
                                 ).then_inc(sof, 16)
                    od += 1
                    half_seq += 1

            for rep in range(reps):
                for s in range(n_spans):
                    gs = rep * n_spans + s
                    if gs >= 1:
                        sy.wait_ge(sin, 32 * gs)  # updater-order: prior ins done
                    if gs >= 2:
                        ps = gs - 2
                        for j, rnd in sorted(span_waits[ps % n_spans].items()):
                            sy.wait_ge(sgr[j], 16 * ((ps // n_spans)
                                                     * slot_tot[j] + rnd))
                        if not gather_only:
                            sy.wait_ge(ss, (ps // n_spans) * n_chunks
                                       + chunks_thru_span[ps % n_spans])
                    c0, cw = span_col0[s], span_cols[s]
                    sy.dma_start(idxb[:, gs % 2, 0:cw * 8],
                                 idx_d[:, c0 * 8:(c0 + cw) * 8]).then_inc(sin, 16)
                    sy.wait_ge(sin, 32 * gs + 16)
                    sy.dma_start(dstfb[:, gs % 2, 0:cw],
                                 dstf_d[:, c0:c0 + cw]).then_inc(sin, 16)
                    if gs >= 2 and not gather_only:
                        ps_s = (gs - 2) % n_spans
                        for t in range(ps_s * GSPAN,
                                       min((ps_s + 1) * GSPAN, NT)):
                            emit_out(t)
            for gs_tr in range(max(0, reps * n_spans - 2), reps * n_spans):
                if gather_only:
                    break
                s = gs_tr % n_spans
                for t in range(s * GSPAN, min((s + 1) * GSPAN, NT)):
                    emit_out(t)
            for j in range(NGSEM):
                if slot_tot[j] > 0:
                    sy.wait_ge(sgr[j], 16 * reps * slot_tot[j])
            if not gather_only:
                sy.wait_ge(sof, 16 * n_valid_halves * reps)

        @block.gpsimd
        def _(g: bass.BassGpSimd):
            g.load_library(mlp)
            g.wait_ge(cio, 16 * (1 + NS))
            cur_span = -1
            waited_pair = -1
            for rep in range(reps):
                for cl in calls:
                    s, b = cl["s"], cl["b"]
                    gs = rep * n_spans + s
                    gpair = rep * n_spans * NB + pair_idx[(s, b)]
                    if gs != cur_span:
                        g.wait_ge(sin, 32 * (gs + 1))
                        cur_span = gs
                    if (gpair >= STGRING and gpair != waited_pair
                            and not gather_only):
                        # slot reuse: block gpair-STGRING released (releases
                        # are span-batched, NB at a time)
                        g.wait_ge(sstg, NB * ((gpair - STGRING) // NB + 1))
                        waited_pair = gpair
                    k_s = rep * slot_tot[cl["slot"]] + cl["round"]
                    if k_s > 1:
                        g.wait_ge(sgr[cl["slot"]], 16 * (k_s - 1))
                    rel = cl["col0"] - blk_col0[(s, b)]
                    srel = cl["col0"] - span_col0[s]
                    n_idx = cl["cols"] * P
                    g.dma_gather(
                        out_ap=stg[:, gpair % STGRING, rel:rel + cl["cols"], :],
                        in_ap=x[b * BANK_ROWS:(b + 1) * BANK_ROWS, :],
                        idxs_ap=idxb[:, gs % 2, srel * 8:(srel + cl["cols"]) * 8],
                        num_idxs=n_idx,
                        num_idxs_reg=n_idx,
                        elem_size=P,
                        single_packet=False,
                        queue_num=cl["q"],
                    ).then_inc(sgr[cl["slot"]], 16)

        @block.vector
        def _(v: bass.BassVectorEngine):
            if gather_only:
                return
            v.wait_ge(cio, 16 * (1 + NS))
            idx = 0
            cur_span = -1
            for rep in range(reps):
                for ch in chunks:
                    gs = rep * n_spans + ch["s"]
                    if gs != cur_span:
                        v.wait_ge(sin, 32 * (gs + 1))
                        cur_span = gs
                    if idx >= SRING and idx % 4 == 0:
                        v.wait_ge(ssf, idx + 4 - SRING)
                    scol = ch["col"] - span_col0[ch["s"]]
                    v.tensor_scalar(
                        out=sring[:, idx % SRING, :], in0=iota_sb[:],
                        scalar1=dstfb[:, gs % 2, scol:scol + 1],
                        scalar2=None, op0=mybir.AluOpType.is_equal,
                    ).then_inc(ss, 1)
                    idx += 1

        @block.tensor
        def _(t_: bass.BassTensorEngine):
            if gather_only:
                return
            t_.wait_ge(cio, 16 * (1 + NS))
            idx = 0
            tile_seq = 0
            half_seq = 0
            pend = []  # tile_seqs awaiting phase2 (depth 2 for ACT slack)

            def phase2(tseq):
                nonlocal half_seq
                t_.wait_ge(sgp, tseq + 1)
                for h in range(2):
                    if half_seq + 1 > OTSLOTS:
                        t_.wait_ge(sot, half_seq + 1 - OTSLOTS)
                    for i in range(NS):
                        mm = t_.matmul(
                            out=op[:, half_seq % OTSLOTS, 0:P],
                            lhsT=gsb[:, tseq % 2, i, h * P:(h + 1) * P],
                            rhs=w_sb[:, i, :],
                            start=(i == 0), stop=(i == NS - 1))
                        if i == NS - 1:
                            mm.then_inc(sp2, 1)
                    half_seq += 1

            for rep in range(reps):
                last_ti = None
                for ch in chunks:
                    s, t, i, b = ch["s"], ch["t"], ch["i"], ch["b"]
                    # first chunk of tile t overall (i == 0 pass)
                    if i == 0 and (t, rep) != last_ti and ch["start"]:
                        if len(pend) >= 2:
                            phase2(pend.pop(0))
                        ts_new = rep * NT + t
                        if ts_new >= GPSLOTS:
                            t_.wait_ge(sgp, ts_new - GPSLOTS + 1)
                        last_ti = (t, rep)
                    if ch["first_of_group"]:
                        for j, rnd in group_waits[(s, b)]:
                            t_.wait_ge(sgr[j], 16 * (rep * slot_tot[j] + rnd))
                    t_.wait_ge(ss, idx + 1)
                    gpair = rep * n_spans * NB + pair_idx[(s, b)]
                    rel = ch["col"] - blk_col0[(s, b)]
                    ts_cur = rep * NT + t
                    mm = t_.matmul(
                        out=gp[:, ts_cur % GPSLOTS, i, :],
                        lhsT=stg[:, gpair % STGRING, rel, :],
                        rhs=sring[:, idx % SRING, :],
                        start=ch["start"], stop=ch["stop"],
                    )
                    mm.then_inc(ssf, 1)
                    idx += 1
                    if idx - rep * n_chunks == tile_chunk_end[t]:
                        t_.drain().then_inc(st, 1)
                        pend.append(tile_seq)
                        tile_seq += 1
                    if idx - rep * n_chunks == chunks_thru_span[s]:
                        # release all NB bank blocks of span s together
                        for _b in range(NB):
                            t_.drain().then_inc(sstg, 1)
                while pend:
                    phase2(pend.pop(0))

        @block.scalar
        def _(a: bass.BassScalarEngine):
            if gather_only:
                return
            tile_seq = 0
            half_seq = 0
            for rep in range(reps):
                for t in range(NT):
                    a.wait_ge(st, tile_seq + 1)
                    if tile_seq >= 2:
                        # gsb slot reuse: both phase2 halves of tile_seq-2 done
                        a.wait_ge(sp2, 2 * (tile_seq - 1))
                    a.activation(out=gsb[:, tile_seq % 2, :, :],
                                 in_=gp[:, tile_seq % GPSLOTS, :, :],
                                 func=mybir.ActivationFunctionType.Copy
                                 ).then_inc(sgp, 1)
                    for h in range(2):
                        a.wait_ge(sp2, half_seq + 1)
                        if half_seq + 1 > OTSLOTS:
                            a.wait_ge(sof, 16 * (half_seq + 1 - OTSLOTS))
                        a.activation(out=ot[:, half_seq % OTSLOTS, :],
                                     in_=op[:, half_seq % OTSLOTS, 0:P],
                                     func=mybir.ActivationFunctionType.Copy
                                     ).then_inc(sot, 1)
                        half_seq += 1
                    tile_seq += 1

    nc.compile()
    return nc


def _make_in_maps(cfg, prep, x, w_list):
    n_nodes = x.shape[0]
    x_pad = np.zeros((cfg.x_rows_pad, P), np.float32)
    x_pad[:n_nodes] = np.asarray(x, np.float32)
    x_pad = x_pad.astype(ml_dtypes.bfloat16)
    iota = np.tile(np.arange(DT, dtype=np.float32)[None, :], (P, 1)).astype(
        ml_dtypes.bfloat16)
    w_cat = np.concatenate([np.asarray(w, np.float32) for w in w_list],
                           axis=0).astype(ml_dtypes.bfloat16)

    return [{
        "x": x_pad,
        "idx": prep["idx_mats"][c],
        "dstf": prep["dstf_mats"][c],
        "iota": iota,
        "w": w_cat,
    } for c in range(N_CORES)]


def kernel(hidden_states, edges_i, edges_ii, edges_iii, edges_a,
           W_i, W_ii, W_iii, W_a):
    x = np.asarray(hidden_states, np.float32)
    cfg = _Cfg(x.shape[0])
    edges_list = [np.asarray(e) for e in (edges_i, edges_ii, edges_iii, edges_a)]
    w_list = [W_i, W_ii, W_iii, W_a]

    prep = _host_prep(cfg, edges_list)
    nc = _build_kernel(cfg, prep)
    in_maps = _make_in_maps(cfg, prep, x, w_list)

    res = run_bass_kernel_spmd(nc, in_maps, core_ids=list(range(N_CORES)))
    out = np.concatenate([np.asarray(res.results[c]["out"], np.float32)
                          for c in range(N_CORES)], axis=0)
    return out.astype(np.float32)


# revision 102
# speedup vs baseline: 1.8410x; 1.1501x over previous
"""GCN message-passing layer (4x GCNConv sum) on 8 Trainium2 NeuronCores.

out[d] = sum_i ( segment_sum_{e in E_i, dst=d} x[src_e] ) @ W_i

Raw-block SPMD kernel (no Tile scheduler): destination nodes sharded across
8 cores, x replicated. Per core:
  - Host groups edges by (core, set, dst-tile(128), src-bank(32768)), sorts
    each group by src (DRAM locality), pads groups to a cross-core-uniform
    multiple of 128 (pad: src=0, dstf=-1).
  - gpsimd dma_gather (int16 bank-local idxs) stages x[src] rows (bf16) into
    SBUF, round-robin over 4 SWDGE queues (4x descriptor throughput).
  - Processing is set-major within each 4-tile span: while PE consumes set
    i's staging, gpsimd prefetches sets i+1, i+2 (stg ring of 3).
  - DVE builds one-hot S[e,d] = (dstf[e]==d) per 128-edge chunk; TensorE
    accumulates g_i^T = stg^T @ S into per-tile PSUM (1 bank, 6-slot ring).
  - ACT copies tile PSUM->SBUF; TensorE phase 2: out_tile = sum_i gsb_i^T.T
    @ W_i into opsum; ACT copies opsum->SBUF; SP DMAs out rows.
All cross-engine sync is explicit semaphores; every core runs an identical
program (counts are cross-core uniform by construction).
"""
import math
import sys
from contextlib import ExitStack

sys.path.insert(0, "/opt/trn_rl_repo")

import numpy as np
import ml_dtypes

from concourse import bass, mybir, bacc
from concourse.bass_utils import run_bass_kernel_spmd
from concourse.library_config import mlp

P = 128
N_CORES = 8
DT = 256           # dst-tile width (two PSUM banks per (tile, 4 sets))
BANK_ROWS = 32768
NS = 4
GSPAN = 2          # tiles per idx/gather span (must be <= GPSLOTS)
CALL_COLS = 96     # max 128-edge chunks per dma_gather call (12288 idxs)
SRING = 8          # one-hot ring slots (multiple of 4: batched ssf waits)
STGRING = 8        # stg (span,bank) block ring slots (> NB)
GPSLOTS = 3        # gpsum PSUM rotation (2 banks each)
OTSLOTS = 2        # opsum/ot half-tile rotation (1 PSUM bank per slot)


class _Cfg:
    def __init__(self, n_nodes):
        self.n_nodes = n_nodes
        self.npc = n_nodes // N_CORES
        self.nt = math.ceil(self.npc / DT)
        self.n_banks = math.ceil(n_nodes / BANK_ROWS)
        self.x_rows_pad = self.n_banks * BANK_ROWS
        self.n_spans = math.ceil(self.nt / GSPAN)


def _host_prep(cfg, edges_list):
    NC, NT, NB = N_CORES, cfg.nt, cfg.n_banks
    counts = np.zeros((NC, NS, NT, NB), np.int64)
    per_set = []
    for i, e in enumerate(edges_list):
        src = np.asarray(e[0], np.int64)
        dst = np.asarray(e[1], np.int64)
        core = dst // cfg.npc
        dloc = dst % cfg.npc
        t = dloc // DT
        b = src // BANK_ROWS
        key = (core * NT + t) * NB + b
        counts[:, i] = np.bincount(key, minlength=NC * NT * NB).reshape(NC, NT, NB)
        order = np.lexsort((src, key))  # src-ascending within each group
        per_set.append((src[order], dloc[order], key[order]))

    C = -(-counts.max(axis=0) // P)
    # guarantee no (set, tile) is entirely empty (phase-2 reads its psum)
    for i in range(NS):
        for t in range(NT):
            if C[i, t].sum() == 0:
                C[i, t, 0] = 1

    col_of = np.zeros((NS, NT, NB), np.int64)
    blk_col0 = {}   # (s, b) -> first col of the bank block
    blk_cols = {}   # (s, b) -> cols in the block (all sets, span tiles)
    span_col0, span_cols = [], []
    col = 0
    for s in range(cfg.n_spans):
        ts = range(s * GSPAN, min((s + 1) * GSPAN, NT))
        span_col0.append(col)
        for b in range(NB):
            blk_col0[(s, b)] = col
            for i in range(NS):
                for t in ts:
                    col_of[i, t, b] = col
                    col += C[i, t, b]
            blk_cols[(s, b)] = col - blk_col0[(s, b)]
        span_cols.append(col - span_col0[-1])
    totc = col
    tot_slots = totc * P

    idx_mats, dstf_mats = [], []
    for c in range(NC):
        slot_src = np.zeros(tot_slots, np.int64)
        slot_dstf = np.full(tot_slots, -1.0, np.float32)
        for i in range(NS):
            src_s, dloc_s, key_s = per_set[i]
            lo = np.searchsorted(key_s, c * NT * NB)
            hi = np.searchsorted(key_s, (c + 1) * NT * NB)
            src_c, dloc_c, key_c = src_s[lo:hi], dloc_s[lo:hi], key_s[lo:hi]
            t_c = (key_c // NB) % NT
            b_c = key_c % NB
            gstart = np.searchsorted(key_c, key_c)
            rank = np.arange(len(key_c)) - gstart
            # lane-major remap: sorted rank r -> slot (r%nch)*128 + r//nch so
            # each SDMA engine's descriptor stream reads consecutive sorted
            # srcs (DRAM row-buffer locality) instead of striding 128 slots.
            nch = C[i, t_c, b_c]
            rank = (rank % nch) * P + rank // nch
            slot = col_of[i, t_c, b_c] * P + rank
            slot_src[slot] = src_c - b_c * BANK_ROWS
            slot_dstf[slot] = (dloc_c - t_c * DT).astype(np.float32)
        idx16 = slot_src.reshape(tot_slots // 16, 16).T.astype(np.int16)
        idx_mats.append(np.tile(idx16, (8, 1)))
        dstf_mats.append(slot_dstf.reshape(totc, P).T.copy())

    # gather call list: per (s,b) bank block, split into <=CALL_COLS chunks
    calls = []
    for s in range(cfg.n_spans):
        for b in range(NB):
            cols = blk_cols[(s, b)]
            if cols == 0:
                continue
            g0 = blk_col0[(s, b)]
            nc_split = -(-cols // CALL_COLS)
            step = -(-cols // nc_split)
            for c0 in range(0, cols, step):
                cw = min(step, cols - c0)
                calls.append(dict(s=s, b=b, col0=g0 + c0, cols=cw))
    # queue = ci % 4 (engine parallelism); completion sem = rotating ring of
    # NGSEM per-call sems (exact completion tracking: cumulative multi-queue
    # counts are unsound because the 16 SDMA engines drain independently).
    NGSEM = 16
    slot_rounds = [0] * NGSEM
    for ci, cl in enumerate(calls):
        cl["q"] = ci % 4
        slot = ci % NGSEM
        slot_rounds[slot] += 1
        cl["slot"] = slot
        cl["round"] = slot_rounds[slot]  # 1-based within rep
    group_waits = {}    # (s,b) -> [(slot, round)] for its calls
    span_waits = {}     # s -> {slot: max round} through end of span
    run = {}
    for cl in calls:
        g = (cl["s"], cl["b"])
        group_waits.setdefault(g, []).append((cl["slot"], cl["round"]))
        run[cl["slot"]] = cl["round"]
        span_waits[cl["s"]] = dict(run)

    return dict(C=C, col_of=col_of, totc=totc,
                blk_col0=blk_col0, blk_cols=blk_cols,
                span_col0=span_col0, span_cols=span_cols,
                idx_mats=idx_mats, dstf_mats=dstf_mats,
                calls=calls, group_waits=group_waits,
                span_waits=span_waits, slot_tot=slot_rounds, ngsem=NGSEM)


def _schedule(cfg, prep):
    """Static per-core schedule: ordered chunk list (set-major within span)
    + per-tile completion chunk counts."""
    C, col_of = prep["C"], prep["col_of"]
    NT, NB = cfg.nt, cfg.n_banks
    chunks = []
    tile_chunk_end = {}   # tile t -> chunk count (1-based) at its completion
    chunks_thru_span = {}
    for s in range(cfg.n_spans):
        ts = range(s * GSPAN, min((s + 1) * GSPAN, NT))
        for i in range(NS):
            for t in ts:
                nib = [(b, int(C[i, t, b])) for b in range(NB) if C[i, t, b] > 0]
                tot_k = sum(n for _, n in nib)
                kk = 0
                for b, cc in nib:
                    for k in range(cc):
                        chunks.append(dict(
                            s=s, t=t, i=i, b=b,
                            col=int(col_of[i, t, b]) + k,
                            start=(kk == 0), stop=(kk == tot_k - 1),
                            first_of_group=(k == 0),
                        ))
                        kk += 1
                if i == NS - 1:
                    tile_chunk_end[t] = len(chunks)
        chunks_thru_span[s] = len(chunks)
    return chunks, tile_chunk_end, chunks_thru_span


def _build_kernel(cfg, prep, reps=1, mode="full"):
    # mode: "full" | "gather" (gathers+input DMAs only, for profiling)
    gather_only = (mode == "gather")
    NT, NB = cfg.nt, cfg.n_banks
    calls = prep["calls"]
    group_waits = prep["group_waits"]
    span_waits = prep["span_waits"]
    slot_tot = prep["slot_tot"]
    NGSEM = prep["ngsem"]
    span_col0, span_cols = prep["span_col0"], prep["span_cols"]
    blk_col0, blk_cols = prep["blk_col0"], prep["blk_cols"]
    col_of = prep["col_of"]
    totc = prep["totc"]
    chunks, tile_chunk_end, chunks_thru_span = _schedule(cfg, prep)

    n_chunks = len(chunks)
    n_calls = len(calls)
    n_spans = cfg.n_spans
    max_span_cols = max(span_cols)
    max_blk_cols = max(blk_cols.values())

    pair_idx = {(s, b): s * NB + b for s in range(n_spans) for b in range(NB)}
    out_rows = {(t, h): max(0, min(P, cfg.npc - (t * DT + h * P)))
                for t in range(NT) for h in range(2)}
    n_valid_halves = sum(1 for k in out_rows if out_rows[k] > 0)

    msg_dt = mybir.dt.bfloat16

    nc = bacc.Bacc("TRN2", target_bir_lowering=False, debug=False,
                   num_devices=N_CORES, num_swdge_queues=4)
    x = nc.dram_tensor("x", [cfg.x_rows_pad, P], msg_dt, kind="ExternalInput").ap()
    idx_d = nc.dram_tensor("idx", [P, totc * 8], mybir.dt.int16,
                           kind="ExternalInput").ap()
    dstf_d = nc.dram_tensor("dstf", [P, totc], mybir.dt.float32,
                            kind="ExternalInput").ap()
    iota_d = nc.dram_tensor("iota", [P, DT], msg_dt, kind="ExternalInput").ap()
    w_d = nc.dram_tensor("w", [NS * P, P], msg_dt,
                         kind="ExternalInput").ap()
    out_d = nc.dram_tensor("out", [cfg.npc, P], msg_dt,
                           kind="ExternalOutput").ap()

    with (
        nc.Block() as block,
        nc.sbuf_tensor("iota_sb", [P, DT], msg_dt) as iota_sb,
        nc.sbuf_tensor("w_sb", [P, NS, P], msg_dt) as w_sb,
        nc.sbuf_tensor("idxb", [P, 2, max_span_cols * 8], mybir.dt.int16) as idxb,
        nc.sbuf_tensor("dstfb", [P, 2, max_span_cols], mybir.dt.float32) as dstfb,
        nc.sbuf_tensor("stg", [P, STGRING, max_blk_cols, P], msg_dt) as stg,
        nc.sbuf_tensor("sring", [P, SRING, DT], msg_dt) as sring,
        nc.sbuf_tensor("gsb", [P, 2, NS, DT], msg_dt) as gsb,
        nc.sbuf_tensor("ot", [P, OTSLOTS, P], msg_dt) as ot,
        nc.psum_tensor("gp", [P, GPSLOTS, NS, DT], mybir.dt.float32) as gp,
        nc.psum_tensor("op", [P, OTSLOTS, 512], mybir.dt.float32) as op,
        ExitStack() as _stack,
    ):
        _sem = lambda n: _stack.enter_context(nc.semaphore(n))
        cio = _sem("cio")    # const loads done (SP)
        sin = _sem("sin")    # span idx+dstf DMA done (SP, +32/span)
        sgr = [_sem(f"sgr{j}") for j in range(NGSEM)]  # per-call rotating
        sstg = _sem("sstg")  # stg bank blocks consumed by PE (+NB/span)
        ss = _sem("ss")      # one-hot ready (DVE, +1/chunk)
        ssf = _sem("ssf")    # one-hot consumed (PE, +1/chunk)
        st = _sem("st")      # tile psum complete (PE, +1/tile)
        sgp = _sem("sgp")    # gsb ready / gpsum freed (ACT, +1/tile)
        sp2 = _sem("sp2")    # opsum ready (PE, +1/tile)
        sot = _sem("sot")    # ot ready (ACT, +1/tile)
        sof = _sem("sof")    # out DMA done (SP, +16/tile)

        @block.sync
        def _(sy: bass.BassEngine):
            sy.dma_start(iota_sb[:], iota_d[:]).then_inc(cio, 16)
            for i in range(NS):
                sy.dma_start(w_sb[:, i, :],
                             w_d[i * P:(i + 1) * P, :]).then_inc(cio, 16)
            half_seq = 0
            od = 0  # completed-out-DMA self-wait counter

            def emit_out(t):
                nonlocal half_seq, od
                for h in range(2):
                    rows = out_rows[(t, h)]
                    if rows <= 0:
                        half_seq += 1
                        continue
                    sy.wait_ge(sot, half_seq + 1)
                    if od > 0:
                        sy.wait_ge(sof, 16 * od)  # updater-order
                    d0 = t * DT + h * P
                    sy.dma_start(out_d[d0:d0 + rows, :],
                                 ot[:rows, half_seq % OTSLOTS, :]
                                 ).then_inc(sof, 16)
                    od += 1
                    half_seq += 1

            for rep in range(reps):
                for s in range(n_spans):
                    gs = rep * n_spans + s
                    if gs >= 1:
                        sy.wait_ge(sin, 32 * gs)  # updater-order: prior ins done
                    if gs >= 2:
                        ps = gs - 2
                        for j, rnd in sorted(span_waits[ps % n_spans].items()):
                            sy.wait_ge(sgr[j], 16 * ((ps // n_spans)
                                                     * slot_tot[j] + rnd))
                        if not gather_only:
                            sy.wait_ge(ss, (ps // n_spans) * n_chunks
                                       + chunks_thru_span[ps % n_spans])
                    c0, cw = span_col0[s], span_cols[s]
                    sy.dma_start(idxb[:, gs % 2, 0:cw * 8],
                                 idx_d[:, c0 * 8:(c0 + cw) * 8]).then_inc(sin, 16)
                    sy.wait_ge(sin, 32 * gs + 16)
                    sy.dma_start(dstfb[:, gs % 2, 0:cw],
                                 dstf_d[:, c0:c0 + cw]).then_inc(sin, 16)
                    if gs >= 2 and not gather_only:
                        ps_s = (gs - 2) % n_spans
                        for t in range(ps_s * GSPAN,
                                       min((ps_s + 1) * GSPAN, NT)):
                            emit_out(t)
            for gs_tr in range(max(0, reps * n_spans - 2), reps * n_spans):
                if gather_only:
                    break
                s = gs_tr % n_spans
                for t in range(s * GSPAN, min((s + 1) * GSPAN, NT)):
                    emit_out(t)
            for j in range(NGSEM):
                if slot_tot[j] > 0:
                    sy.wait_ge(sgr[j], 16 * reps * slot_tot[j])
            if not gather_only:
                sy.wait_ge(sof, 16 * n_valid_halves * reps)

        @block.gpsimd
        def _(g: bass.BassGpSimd):
            g.load_library(mlp)
            g.wait_ge(cio, 16 * (1 + NS))
            cur_span = -1
            waited_pair = -1
            for rep in range(reps):
                for cl in calls:
                    s, b = cl["s"], cl["b"]
                    gs = rep * n_spans + s
                    gpair = rep * n_spans * NB + pair_idx[(s, b)]
                    if gs != cur_span:
                        g.wait_ge(sin, 32 * (gs + 1))
                        cur_span = gs
                    if (gpair >= STGRING and gpair != waited_pair
                            and not gather_only):
                        # slot reuse: block gpair-STGRING released (releases
                        # are span-batched, NB at a time)
                        g.wait_ge(sstg, NB * ((gpair - STGRING) // NB + 1))
                        waited_pair = gpair
                    k_s = rep * slot_tot[cl["slot"]] + cl["round"]
                    if k_s > 1:
                        g.wait_ge(sgr[cl["slot"]], 16 * (k_s - 1))
                    rel = cl["col0"] - blk_col0[(s, b)]
                    srel = cl["col0"] - span_col0[s]
                    n_idx = cl["cols"] * P
                    g.dma_gather(
                        out_ap=stg[:, gpair % STGRING, rel:rel + cl["cols"], :],
                        in_ap=x[b * BANK_ROWS:(b + 1) * BANK_ROWS, :],
                        idxs_ap=idxb[:, gs % 2, srel * 8:(srel + cl["cols"]) * 8],
                        num_idxs=n_idx,
                        num_idxs_reg=n_idx,
                        elem_size=P,
                        single_packet=False,
                        queue_num=cl["q"],
                    ).then_inc(sgr[cl["slot"]], 16)

        @block.vector
        def _(v: bass.BassVectorEngine):
            if gather_only:
                return
            v.wait_ge(cio, 16 * (1 + NS))
            idx = 0
            cur_span = -1
            for rep in range(reps):
                for ch in chunks:
                    gs = rep * n_spans + ch["s"]
                    if gs != cur_span:
                        v.wait_ge(sin, 32 * (gs + 1))
                        cur_span = gs
                    if idx >= SRING and idx % 4 == 0:
                        v.wait_ge(ssf, idx + 4 - SRING)
                    scol = ch["col"] - span_col0[ch["s"]]
                    v.tensor_scalar(
                        out=sring[:, idx % SRING, :], in0=iota_sb[:],
                        scalar1=dstfb[:, gs % 2, scol:scol + 1],
                        scalar2=None, op0=mybir.AluOpType.is_equal,
                    ).then_inc(ss, 1)
                    idx += 1

        @block.tensor
        def _(t_: bass.BassTensorEngine):
            if gather_only:
                return
            t_.wait_ge(cio, 16 * (1 + NS))
            idx = 0
            tile_seq = 0
            half_seq = 0
            pend = []  # tile_seqs awaiting phase2 (depth 2 for ACT slack)

            def phase2(tseq):
                nonlocal half_seq
                t_.wait_ge(sgp, tseq + 1)
                for h in range(2):
                    if half_seq + 1 > OTSLOTS:
                        t_.wait_ge(sot, half_seq + 1 - OTSLOTS)
                    for i in range(NS):
                        mm = t_.matmul(
                            out=op[:, half_seq % OTSLOTS, 0:P],
                            lhsT=gsb[:, tseq % 2, i, h * P:(h + 1) * P],
                            rhs=w_sb[:, i, :],
                            start=(i == 0), stop=(i == NS - 1))
                        if i == NS - 1:
                            mm.then_inc(sp2, 1)
                    half_seq += 1

            for rep in range(reps):
                last_ti = None
                for ch in chunks:
                    s, t, i, b = ch["s"], ch["t"], ch["i"], ch["b"]
                    # first chunk of tile t overall (i == 0 pass)
                    if i == 0 and (t, rep) != last_ti and ch["start"]:
                        if len(pend) >= 2:
                            phase2(pend.pop(0))
                        ts_new = rep * NT + t
                        if ts_new >= GPSLOTS:
                            t_.wait_ge(sgp, ts_new - GPSLOTS + 1)
                        last_ti = (t, rep)
                    if ch["first_of_group"]:
                        for j, rnd in group_waits[(s, b)]:
                            t_.wait_ge(sgr[j], 16 * (rep * slot_tot[j] + rnd))
                    t_.wait_ge(ss, idx + 1)
                    gpair = rep * n_spans * NB + pair_idx[(s, b)]
                    rel = ch["col"] - blk_col0[(s, b)]
                    ts_cur = rep * NT + t
                    mm = t_.matmul(
                        out=gp[:, ts_cur % GPSLOTS, i, :],
                        lhsT=stg[:, gpair % STGRING, rel, :],
                        rhs=sring[:, idx % SRING, :],
                        start=ch["start"], stop=ch["stop"],
                    )
                    mm.then_inc(ssf, 1)
                    idx += 1
                    if idx - rep * n_chunks == tile_chunk_end[t]:
                        t_.drain().then_inc(st, 1)
                        pend.append(tile_seq)
                        tile_seq += 1
                    if idx - rep * n_chunks == chunks_thru_span[s]:
                        # release all NB bank blocks of span s together
                        for _b in range(NB):
                            t_.drain().then_inc(sstg, 1)
                while pend:
                    phase2(pend.pop(0))

        @block.scalar
        def _(a: bass.BassScalarEngine):
            if gather_only:
                return
            tile_seq = 0
            half_seq = 0
            for rep in range(reps):
                for t in range(NT):
                    a.wait_ge(st, tile_seq + 1)
                    if tile_seq >= 2:
                        # gsb slot reuse: both phase2 halves of tile_seq-2 done
                        a.wait_ge(sp2, 2 * (tile_seq - 1))
                    a.activation(out=gsb[:, tile_seq % 2, :, :],
                                 in_=gp[:, tile_seq % GPSLOTS, :, :],
                                 func=mybir.ActivationFunctionType.Copy
                                 ).then_inc(sgp, 1)
                    for h in range(2):
                        a.wait_ge(sp2, half_seq + 1)
                        if half_seq + 1 > OTSLOTS:
                            a.wait_ge(sof, 16 * (half_seq + 1 - OTSLOTS))
                        a.activation(out=ot[:, half_seq % OTSLOTS, :],
                                     in_=op[:, half_seq % OTSLOTS, 0:P],
                                     func=mybir.ActivationFunctionType.Copy
                                     ).then_inc(sot, 1)
                        half_seq += 1
                    tile_seq += 1

    nc.compile()
    return nc


def _make_in_maps(cfg, prep, x, w_list):
    n_nodes = x.shape[0]
    x_pad = np.zeros((cfg.x_rows_pad, P), np.float32)
    x_pad[:n_nodes] = np.asarray(x, np.float32)
    x_pad = x_pad.astype(ml_dtypes.bfloat16)
    iota = np.tile(np.arange(DT, dtype=np.float32)[None, :], (P, 1)).astype(
        ml_dtypes.bfloat16)
    w_cat = np.concatenate([np.asarray(w, np.float32) for w in w_list],
                           axis=0).astype(ml_dtypes.bfloat16)

    return [{
        "x": x_pad,
        "idx": prep["idx_mats"][c],
        "dstf": prep["dstf_mats"][c],
        "iota": iota,
        "w": w_cat,
    } for c in range(N_CORES)]


def kernel(hidden_states, edges_i, edges_ii, edges_iii, edges_a,
           W_i, W_ii, W_iii, W_a):
    x = np.asarray(hidden_states, np.float32)
    cfg = _Cfg(x.shape[0])
    edges_list = [np.asarray(e) for e in (edges_i, edges_ii, edges_iii, edges_a)]
    w_list = [W_i, W_ii, W_iii, W_a]

    prep = _host_prep(cfg, edges_list)
    nc = _build_kernel(cfg, prep)
    in_maps = _make_in_maps(cfg, prep, x, w_list)

    res = run_bass_kernel_spmd(nc, in_maps, core_ids=list(range(N_CORES)))
    out = np.concatenate([np.asarray(res.results[c]["out"], np.float32)
                          for c in range(N_CORES)], axis=0)
    return out.astype(np.float32)
